# revision 7
# baseline (speedup 1.0000x reference)
"""Trainium2 Bass kernel for nn_ActSeries: 20 layers of per-channel range-norm +
quadratic polynomial, x [32,32,256,256] f32.

Strategy (v2 — analytic range propagation, dual-engine streaming)
-----------------------------------------------------------------
Shard the 32 channels across 8 cores (4 channels/core); per-channel stats make
every reduction core-local (no collectives).

Math: each layer is h' = a2*xh^2 + a1*xh + a0 with xh = (h-mn)/(mx-mn+eps).
Complete the square: h' = a2*(xh + d2)^2 + const, d2 = a1/(2*a2). The range-norm
is invariant to tracked affine maps, so we store Z = gamma*xh + delta and fold
each layer into Z' = (alpha*Z + beta)^2 (one multiply-add-square per element).
Key observation: the data min/max of the NEXT layer is analytic given this
layer's range [0, A]: max over the interval is attained at an endpoint (both
endpoints ARE data points), and the interior-vertex min is ~0 to within the
data spacing squared (~1e-12), far below the 2e-2 tolerance. So after a single
min/max scan of the raw input (layer 0), all 20 layers' scale/offset constants
follow from a tiny per-channel scalar recurrence — no more data scans, no
inter-layer dependencies beyond the elementwise stream.

Per-pair affine normalization: the A-layer (even) picks its output scale
w = sqrt(|a2*s'|) so gamma_mid = +-1; the B-layer (odd) then needs no scale:
Z'' = (Z' + betab)^2. Two layers fuse into ONE 5-stage custom DVE op
  out = sq(sq(Src0*C0 + C1) + C3)   (C0=alpha, C1=beta, C3=betab via Src1 latch)
at 1 elem/cycle, i.e. 2 layer-elements/cycle. The Scalar engine computes the
same layers via ACTIVATE Square ((scale*x+bias)^2), so DVE and ACT split the
chunks ~5:3 and run concurrently. Final y = cf1*Z + cf0 in one affine pass.
Everything runs in place (verified on HW); 3 channel buffers rotate in SBUF.

Validated end-to-end in numpy against the reference: rel err ~2e-4.
"""

import os
import sys

import numpy as np

B, C, H, Wd = 32, 32, 256, 256
N_LAYERS = 20
N_PAIRS = N_LAYERS // 2
EPS = 1e-5
N_CORES = 8
CH_PER_CORE = C // N_CORES  # 4
F_FULL = B * H * Wd // 128  # 16384 free-dim elements per partition
CW = 4096
NCHUNK = F_FULL // CW  # 4
CLAMP = 1e-4  # |a2| clamp; error bounded by CLAMP*A^2 << tol

# Engine assignment: channels 0-2 chunk-static (ACT takes chunks {1,3});
# channel 3 (the deferred-buffer channel) is pair-level mixed so both engines
# share its tail. Ratio tuned for DVE pair-op 4.54us vs ACT 2-activate 7.0us.
def unit_engine_is_act(c, k, p):
    if c < 3:
        return k in (1, 3)
    return (p * 4 + k) % 8 >= 5


def affine_engine_is_act(c, k):
    if c < 3:
        return k in (1, 3)
    return k in (1, 3)

# coef column layout: 8 per-layer arrays of [N_LAYERS*4] (l*4+c), then cf0 [4]
_NL4 = N_LAYERS * CH_PER_CORE  # 80
_COEF_NAMES = ("d2", "e0", "nf", "g", "absa2", "r_a2", "sgn", "r_absa2")
NCOEF = len(_COEF_NAMES) * _NL4 + CH_PER_CORE  # 644


def _import_concourse():
    try:
        import concourse  # noqa: F401
    except ImportError:
        for p in ("/opt/trn_rl_repo", os.path.expanduser("~/.axon_site/_ro/trn_rl_repo")):
            if os.path.isdir(p) and p not in sys.path:
                sys.path.insert(0, p)
        import concourse  # noqa: F401


def register_pair_op():
    """out = sq(sq(Src0*C0 + C1) + C3): two fused layers, C3 spilled to Src1."""
    _import_concourse()
    from concourse import dve_ops as dvo
    from concourse.dve_spec import (
        C0,
        C1,
        C3,
        Spec,
        Src0,
        _has_src1,
        _spill_c3_to_src1,
        lower,
        sq,
    )
    from concourse.dve_uop import DveOpSpec

    name = "SQ_PAIR_ANT"
    for op in dvo.OPS:
        if op.name == name:
            return op

    def _ref(in0, in1, s0, s1, imm2):
        x = in0.astype(np.float32)
        bb = np.asarray(in1, dtype=np.float32).reshape(x.shape[0], -1)[:, :1]
        v = (x * s0 + s1).astype(np.float32)
        o1 = (v * v).astype(np.float32)
        v2 = (o1 + bb).astype(np.float32)
        return (v2 * v2).astype(np.float32)

    body = _spill_c3_to_src1(sq(sq(Src0 * C0 + C1) + C3))
    spec = Spec(body=body, reference=_ref)
    row = max(dvo._SUB_OPCODE_FOR_NAME.values()) + 1
    uops = lower(spec, ver="v3")
    sha = DveOpSpec(name=name, opcode=row, uops=uops, rd1_en=_has_src1(spec)).sha("v3")
    op = dvo.DveOp(name=name, spec=spec, subdim=False, uops_sha={"v3": sha})
    dvo.OPS.append(op)
    dvo._SUB_OPCODE_FOR_NAME[name] = row
    dvo.CUSTOM_DVE_SPECS[name] = spec
    return op


def register_scan_op(name, alu_name, init_name):
    """f32 min/max scan with accum, with an authored 2X_2P perf variant
    (2 elem/cycle via both SBUF read ports; mirrors the stock tensor_scalar
    2X_2P control conventions). out = in (passthrough), accum_out = min/max."""
    import copy as _copy

    _import_concourse()
    from concourse import dve_ops as dvo
    from concourse.dve_spec import Leaf, Spec, Src0, lower
    from concourse.dve_uop import AluInp, AluOp as UAlu, DveOpSpec, InpSel, OutPath, OutSel

    for op in dvo.OPS:
        if op.name == name:
            return op
    alu = getattr(UAlu, alu_name)
    init_sel = getattr(InpSel, init_name)
    spec = Spec(body=Src0, accum=alu, accum_init=Leaf(init_sel))
    uops1x = lower(spec, ver="v3")
    assert len(uops1x) == 2
    seed2p = _copy.deepcopy(uops1x[0])
    st = _copy.deepcopy(uops1x[1])
    st.enable_input(InpSel.SRC_1, 3)  # second stream on lane 2
    st.require_inp1 = 1
    for b in st.datapath_config:
        b.pass_through_delay(2)
    st.datapath_config[0].enable_alu(alu, AluInp.PREV_DELAY_0, AluInp.PREV_DELAY_2)
    st.enable_output(OutSel.DELAY_2, OutPath.WR1_LO)
    uops2p = [seed2p, st]
    row = max(dvo._SUB_OPCODE_FOR_NAME.values()) + 1
    dspec = DveOpSpec(
        name=name,
        opcode=row,
        uops=uops1x,
        uops_2x=uops2p,
        uops_2x_2p=uops2p,
        uops_4x=None,
        perf_max=2,
        rd1_en=False,
    )
    op = dvo.DveOp(name=name, spec=spec, subdim=False, uops_sha={"v3": dspec.sha("v3")})
    dvo.OPS.append(op)
    dvo._SUB_OPCODE_FOR_NAME[name] = row
    dvo.CUSTOM_DVE_SPECS[name] = spec
    dvo._COMPILE_CACHE[(name, "v3")] = dspec
    return op


def build_nc(enable_asserts=False):
    _import_concourse()
    import concourse.bacc as bacc
    import concourse.tile as tile
    from concourse import bass_isa, mybir

    pair_op = register_pair_op()
    scan_min = register_scan_op("SCAN_MIN_2P_ANT", "MIN", "MAX_POS")
    scan_max = register_scan_op("SCAN_MAX_2P_ANT", "MAX", "MAX_NEG")

    f32 = mybir.dt.float32
    Alu = mybir.AluOpType
    Act = mybir.ActivationFunctionType
    AX = mybir.AxisListType

    nc = bacc.Bacc(
        "TRN2",
        target_bir_lowering=False,
        debug=False,
        enable_asserts=enable_asserts,
        num_devices=N_CORES,
    )

    xs = nc.dram_tensor("xs", [CH_PER_CORE, 128, F_FULL], f32, kind="ExternalInput").ap()
    coef = nc.dram_tensor("coef", [128, NCOEF], f32, kind="ExternalInput").ap()
    ys = nc.dram_tensor("ys", [CH_PER_CORE, 128, F_FULL], f32, kind="ExternalOutput").ap()

    with tile.TileContext(nc) as tc:
        with (
            tc.tile_pool(name="data", bufs=3) as dpool,
            tc.tile_pool(name="cst", bufs=1) as cpool,
            tc.tile_pool(name="st", bufs=2) as st,
            tc.tile_pool(name="pt", bufs=4) as pt,
        ):
            coeft = cpool.tile([128, NCOEF], f32, tag="coeft", name="coeft")
            nc.sync.dma_start(out=coeft[:], in_=coef)

            def cv(nm, l):
                base = _COEF_NAMES.index(nm) * _NL4 + l * CH_PER_CORE
                return coeft[:, base : base + CH_PER_CORE]

            cf0v = coeft[:, len(_COEF_NAMES) * _NL4 :]

            alphaT = cpool.tile([128, N_PAIRS * 4], f32, tag="alphaT", name="alphaT")
            betaT = cpool.tile([128, N_PAIRS * 4], f32, tag="betaT", name="betaT")
            betabT = cpool.tile([128, N_PAIRS * 4], f32, tag="betabT", name="betabT")
            cf1T = cpool.tile([128, 4], f32, tag="cf1T", name="cf1T")
            mn0t = cpool.tile([128, 4], f32, tag="mn0t", name="mn0t")
            mx0t = cpool.tile([128, 4], f32, tag="mx0t", name="mx0t")

            def s4(tag):
                return st.tile([128, 4], f32, tag=tag, name=tag)

            # ---------- Phase 1: DMA in + layer-0 min/max scans ----------
            def scan_chunk(src_chunk, c, k, pmn, pmx):
                i1 = nc.vector._custom_dve(
                    scan_min, out=src_chunk, in0=src_chunk, accum_out=pmn[:, k : k + 1]
                )
                i1.perf_max = 2
                i2 = nc.vector._custom_dve(
                    scan_max, out=src_chunk, in0=src_chunk, accum_out=pmx[:, k : k + 1]
                )
                i2.perf_max = 2

            def combine(c, pmn, pmx):
                rmn = pt.tile([128, 1], f32, tag="rmn", name="rmn")
                rmx = pt.tile([128, 1], f32, tag="rmx", name="rmx")
                nc.vector.tensor_reduce(rmn[:], pmn[:], axis=AX.X, op=Alu.min)
                nc.vector.tensor_reduce(rmx[:], pmx[:], axis=AX.X, op=Alu.max)
                nc.vector.tensor_scalar_mul(rmn[:], rmn[:], -1.0)
                nmn = pt.tile([128, 1], f32, tag="nmn", name="nmn")
                nc.gpsimd.partition_all_reduce(nmn[:], rmn[:], 128, bass_isa.ReduceOp.max)
                nc.vector.tensor_scalar_mul(mn0t[:, c : c + 1], nmn[:], -1.0)
                nc.gpsimd.partition_all_reduce(
                    mx0t[:, c : c + 1], rmx[:], 128, bass_isa.ReduceOp.max
                )

            # ch3 stream-scanned via two chunk-scratch acquisitions (slots 0,1)
            scr = [
                dpool.tile([128, CW], f32, tag="W", name=f"scr{i}") for i in range(2)
            ]
            pmn3 = pt.tile([128, NCHUNK], f32, tag="pmn", name="pmn3")
            pmx3 = pt.tile([128, NCHUNK], f32, tag="pmx", name="pmx3")
            for k in range(NCHUNK):
                s = scr[k % 2]
                nc.sync.dma_start(out=s[:], in_=xs[3][:, k * CW : (k + 1) * CW])
                scan_chunk(s[:], 3, k, pmn3, pmx3)

            W = {}
            for c in range(3):
                W[c] = dpool.tile([128, F_FULL], f32, tag="W", name=f"W{c}")
                pmn = pt.tile([128, NCHUNK], f32, tag="pmn", name=f"pmn{c}")
                pmx = pt.tile([128, NCHUNK], f32, tag="pmx", name=f"pmx{c}")
                for k in range(NCHUNK):
                    ck = W[c][:, k * CW : (k + 1) * CW]
                    nc.sync.dma_start(out=ck, in_=xs[c][:, k * CW : (k + 1) * CW])
                    scan_chunk(ck, c, k, pmn, pmx)
                combine(c, pmn, pmx)
            combine(3, pmn3, pmx3)

            # ---------- Phase 2+3 interleaved: chain (1 pair lookahead) + units
            # boot
            D0 = s4("D0")
            nc.vector.tensor_sub(D0[:], mx0t[:], mn0t[:])
            Dse0 = s4("Dse0")
            nc.vector.tensor_scalar_add(Dse0[:], D0[:], EPS)
            sp0 = s4("sp0")
            nc.vector.reciprocal(sp0[:], Dse0[:])
            A = s4("A")
            nc.vector.tensor_scalar(A[:], sp0[:], -EPS, 1.0, Alu.mult, Alu.add)
            rgamma = sp0
            delta = mn0t

            state = {"A": A, "rgamma": rgamma, "delta": delta, "gmid": None, "dmid": None}

            def chain_layer_stats(l):
                t1 = s4("t1")
                nc.vector.tensor_add(t1[:], state["A"][:], cv("d2", l))
                eA = s4("eA")
                nc.vector.tensor_mul(eA[:], t1[:], t1[:])
                i_ = s4("i_")
                nc.vector.scalar_tensor_tensor(
                    i_[:], t1[:], 0.0, cv("nf", l), Alu.is_gt, Alu.mult
                )
                j = s4("j")
                nc.vector.tensor_scalar(j[:], i_[:], -1.0, 1.0, Alu.mult, Alu.add)
                mne = s4("mne")
                nc.vector.tensor_tensor(mne[:], eA[:], cv("e0", l), Alu.min)
                mn = s4("mn")
                nc.vector.tensor_mul(mn[:], mne[:], j[:])
                mx = s4("mx")
                nc.vector.tensor_tensor(mx[:], eA[:], cv("e0", l), Alu.max)
                spr = s4("spr")
                nc.vector.tensor_sub(spr[:], mx[:], mn[:])
                tg = s4("tg")
                nc.vector.tensor_mul(tg[:], spr[:], cv("g", l))
                E = s4("E")
                nc.vector.tensor_sub(E[:], mx[:], tg[:])
                tD = s4("tD")
                nc.vector.tensor_mul(tD[:], spr[:], cv("absa2", l))
                Dse = s4("Dse")
                nc.vector.tensor_scalar_add(Dse[:], tD[:], EPS)
                sp = s4("sp")
                nc.vector.reciprocal(sp[:], Dse[:])
                Anew = s4("Anew")
                nc.vector.tensor_scalar(Anew[:], sp[:], -EPS, 1.0, Alu.mult, Alu.add)
                state["A"] = Anew
                return E, Dse, sp

            def chain_pair(p):
                lA, lB = 2 * p, 2 * p + 1
                av = alphaT[:, p * 4 : p * 4 + 4]
                bv = betaT[:, p * 4 : p * 4 + 4]
                bbv = betabT[:, p * 4 : p * 4 + 4]
                # A-layer
                E, Dse, sp = chain_layer_stats(lA)
                aspa = s4("aspa")
                nc.vector.tensor_mul(aspa[:], cv("absa2", lA), sp[:])
                w = s4("w")
                nc.scalar.activation(w[:], aspa[:], Act.Sqrt)
                w2 = s4("w2")
                nc.vector.tensor_mul(w2[:], w[:], w[:])
                raspa = s4("raspa")
                nc.vector.tensor_mul(raspa[:], Dse[:], cv("r_absa2", lA))
                gmu = s4("gmu")
                nc.vector.tensor_mul(gmu[:], w2[:], raspa[:])
                gmid = s4("gmid")
                nc.vector.tensor_mul(gmid[:], gmu[:], cv("sgn", lA))
                nc.vector.tensor_mul(av, w[:], state["rgamma"][:])
                tad = s4("tad")
                nc.vector.tensor_mul(tad[:], av, state["delta"][:])
                twd = s4("twd")
                nc.vector.tensor_mul(twd[:], w[:], cv("d2", lA))
                nc.vector.tensor_sub(bv, twd[:], tad[:])
                dmid = s4("dmid")
                nc.vector.tensor_mul(dmid[:], w2[:], E[:])
                # B-layer
                E2, Dse2, _sp2 = chain_layer_stats(lB)
                tbd = s4("tbd")
                nc.vector.tensor_mul(tbd[:], gmid[:], cv("d2", lB))
                nc.vector.tensor_sub(bbv, tbd[:], dmid[:])
                gm2 = s4("gm2")
                nc.vector.tensor_mul(gm2[:], gmid[:], gmid[:])
                if p < N_PAIRS - 1:
                    tg2 = s4("tg2")
                    nc.vector.tensor_mul(tg2[:], gm2[:], Dse2[:])
                    gam = s4("gam")
                    nc.vector.tensor_mul(gam[:], tg2[:], cv("r_a2", lB))
                    dele = s4("dele")
                    nc.vector.tensor_mul(dele[:], gm2[:], E2[:])
                    rg = s4("rg")
                    nc.vector.reciprocal(rg[:], gam[:])
                    state["rgamma"] = rg
                    state["delta"] = dele
                else:
                    rgm2 = s4("rgm2")
                    nc.vector.reciprocal(rgm2[:], gm2[:])
                    # cf1 = a2cl / gm2 ; a2cl = absa2*sgn
                    a2c = s4("a2c")
                    nc.vector.tensor_mul(a2c[:], cv("absa2", lB), cv("sgn", lB))
                    nc.vector.tensor_mul(cf1T[:], a2c[:], rgm2[:])

            def unit(c, k, p):
                ck = W[c][:, k * CW : (k + 1) * CW]
                a_ap = alphaT[:, p * 4 + c : p * 4 + c + 1]
                b_ap = betaT[:, p * 4 + c : p * 4 + c + 1]
                bb_ap = betabT[:, p * 4 + c : p * 4 + c + 1]
                if unit_engine_is_act(c, k, p):
                    nc.scalar.activation(ck, ck, Act.Square, bias=b_ap, scale=a_ap)
                    nc.scalar.activation(ck, ck, Act.Square, bias=bb_ap, scale=1.0)
                else:
                    nc.vector._custom_dve(
                        pair_op, out=ck, in0=ck, in1=bb_ap, s0=a_ap, s1=b_ap
                    )

            def finish_chunk(c, k):
                ck = W[c][:, k * CW : (k + 1) * CW]
                cf1_ap = cf1T[:, c : c + 1]
                cf0_ap = cf0v[:, c : c + 1]
                if affine_engine_is_act(c, k):
                    nc.scalar.activation(ck, ck, Act.Identity, bias=cf0_ap, scale=cf1_ap)
                else:
                    nc.vector.tensor_scalar(ck, ck, cf1_ap, cf0_ap, Alu.mult, Alu.add)
                nc.sync.dma_start(out=ys[c][:, k * CW : (k + 1) * CW], in_=ck)

            # chain runs 1 pair ahead of the units of channels 0-2
            chain_pair(0)
            for p in range(N_PAIRS):
                if p + 1 < N_PAIRS:
                    chain_pair(p + 1)
                for c in range(3):
                    for k in range(NCHUNK):
                        unit(c, k, p)
            for c in range(3):
                for k in range(NCHUNK):
                    finish_chunk(c, k)

            # ---------- Phase 3b: channel 3 (buffer freed by ch0) ----------
            W[3] = dpool.tile([128, F_FULL], f32, tag="W", name="W3")
            for k in range(NCHUNK):
                nc.sync.dma_start(
                    out=W[3][:, k * CW : (k + 1) * CW],
                    in_=xs[3][:, k * CW : (k + 1) * CW],
                )
            for k in range(NCHUNK):
                for p in range(N_PAIRS):
                    unit(3, k, p)
                finish_chunk(3, k)

    nc.compile()
    return nc


_NC_CACHE = {}


def _get_nc():
    if "full" not in _NC_CACHE:
        _NC_CACHE["full"] = build_nc()
    return _NC_CACHE["full"]


def host_coefs(w0, w1, w2):
    """Per-core coef arrays [128, NCOEF] (f32, broadcast over partitions)."""
    f = np.float32
    a2 = np.asarray(w2, dtype=f)
    a1 = np.asarray(w1, dtype=f)
    a0 = np.asarray(w0, dtype=f)
    sgn = np.where(a2 >= 0, f(1), f(-1)).astype(f)
    a2cl = (sgn * np.maximum(np.abs(a2), f(CLAMP))).astype(f)
    d2 = (a1 / a2cl / 2).astype(f)
    e0 = (d2 * d2).astype(f)
    nf = (d2 < 0).astype(f)
    g = (a2cl >= 0).astype(f)
    absa2 = np.abs(a2cl).astype(f)
    r_a2 = (f(1) / a2cl).astype(f)
    r_absa2 = (f(1) / absa2).astype(f)
    arrays = {
        "d2": d2, "e0": e0, "nf": nf, "g": g,
        "absa2": absa2, "r_a2": r_a2, "sgn": sgn, "r_absa2": r_absa2,
    }
    cf0 = (a0[N_LAYERS - 1] - a2cl[N_LAYERS - 1] * e0[N_LAYERS - 1]).astype(f)

    out = []
    for core in range(N_CORES):
        cols = slice(CH_PER_CORE * core, CH_PER_CORE * (core + 1))
        row = np.empty(NCOEF, dtype=f)
        for idx, nm in enumerate(_COEF_NAMES):
            arr = arrays[nm][:, cols]  # [NL, 4]
            row[idx * _NL4 : (idx + 1) * _NL4] = arr.reshape(-1)  # l*4+c
        row[len(_COEF_NAMES) * _NL4 :] = cf0[cols]
        out.append(np.ascontiguousarray(np.broadcast_to(row[None, :], (128, NCOEF))))
    return out


def shard_inputs(x, w0, w1, w2):
    x = np.ascontiguousarray(x, dtype=np.float32)
    coefs = host_coefs(w0, w1, w2)
    in_maps = []
    for k in range(N_CORES):
        cols = slice(CH_PER_CORE * k, CH_PER_CORE * (k + 1))
        xk = np.ascontiguousarray(x[:, cols].transpose(1, 0, 2, 3)).reshape(
            CH_PER_CORE, 128, F_FULL
        )
        in_maps.append({"xs": xk, "coef": coefs[k]})
    return in_maps


def unshard_output(results):
    out = np.empty((B, C, H, Wd), dtype=np.float32)
    for k in range(N_CORES):
        ysk = np.asarray(results[k]["ys"], dtype=np.float32).reshape(
            CH_PER_CORE, B, H, Wd
        )
        out[:, CH_PER_CORE * k : CH_PER_CORE * (k + 1)] = ysk.transpose(1, 0, 2, 3)
    return out


def run_sharded(in_maps, trace=False, trace_kwargs=None):
    _import_concourse()
    from concourse.bass_utils import run_bass_kernel_spmd

    nc = _get_nc()
    return run_bass_kernel_spmd(
        nc,
        in_maps,
        core_ids=list(range(N_CORES)),
        trace=trace,
        **(trace_kwargs or {}),
    )


def kernel(x, w0, w1, w2):
    in_maps = shard_inputs(x, w0, w1, w2)
    res = run_sharded(in_maps)
    return unshard_output(res.results)


# revision 8
# speedup vs baseline: 1.0129x; 1.0129x over previous
"""Trainium2 Bass kernel for nn_ActSeries: 20 layers of per-channel range-norm +
quadratic polynomial, x [32,32,256,256] f32.

Strategy (v2 — analytic range propagation, dual-engine streaming)
-----------------------------------------------------------------
Shard the 32 channels across 8 cores (4 channels/core); per-channel stats make
every reduction core-local (no collectives).

Math: each layer is h' = a2*xh^2 + a1*xh + a0 with xh = (h-mn)/(mx-mn+eps).
Complete the square: h' = a2*(xh + d2)^2 + const, d2 = a1/(2*a2). The range-norm
is invariant to tracked affine maps, so we store Z = gamma*xh + delta and fold
each layer into Z' = (alpha*Z + beta)^2 (one multiply-add-square per element).
Key observation: the data min/max of the NEXT layer is analytic given this
layer's range [0, A]: max over the interval is attained at an endpoint (both
endpoints ARE data points), and the interior-vertex min is ~0 to within the
data spacing squared (~1e-12), far below the 2e-2 tolerance. So after a single
min/max scan of the raw input (layer 0), all 20 layers' scale/offset constants
follow from a tiny per-channel scalar recurrence — no more data scans, no
inter-layer dependencies beyond the elementwise stream.

Per-pair affine normalization: the A-layer (even) picks its output scale
w = sqrt(|a2*s'|) so gamma_mid = +-1; the B-layer (odd) then needs no scale:
Z'' = (Z' + betab)^2. Two layers fuse into ONE 5-stage custom DVE op
  out = sq(sq(Src0*C0 + C1) + C3)   (C0=alpha, C1=beta, C3=betab via Src1 latch)
at 1 elem/cycle, i.e. 2 layer-elements/cycle. The Scalar engine computes the
same layers via ACTIVATE Square ((scale*x+bias)^2), so DVE and ACT split the
chunks ~5:3 and run concurrently. Final y = cf1*Z + cf0 in one affine pass.
Everything runs in place (verified on HW); 3 channel buffers rotate in SBUF.

Validated end-to-end in numpy against the reference: rel err ~2e-4.
"""

import os
import sys

import numpy as np

B, C, H, Wd = 32, 32, 256, 256
N_LAYERS = 20
N_PAIRS = N_LAYERS // 2
EPS = 1e-5
N_CORES = 8
CH_PER_CORE = C // N_CORES  # 4
F_FULL = B * H * Wd // 128  # 16384 free-dim elements per partition
CW = 4096
NCHUNK = F_FULL // CW  # 4
CLAMP = 1e-4  # |a2| clamp; error bounded by CLAMP*A^2 << tol

# Engine assignment: channels 0-2 chunk-static (ACT takes chunks {1,3});
# channel 3 (the deferred-buffer channel) is pair-level mixed so both engines
# share its tail. Ratio tuned for DVE pair-op 4.54us vs ACT 2-activate 7.0us.
def unit_engine_is_act(c, k, p):
    return k in (1, 3)


def affine_engine_is_act(c, k):
    return k in (1, 3)

# coef column layout: 8 per-layer arrays of [N_LAYERS*4] (l*4+c), then cf0 [4]
_NL4 = N_LAYERS * CH_PER_CORE  # 80
_COEF_NAMES = ("d2", "e0", "nf", "g", "absa2", "r_a2", "sgn", "r_absa2")
NCOEF = len(_COEF_NAMES) * _NL4 + CH_PER_CORE  # 644


def _import_concourse():
    try:
        import concourse  # noqa: F401
    except ImportError:
        for p in ("/opt/trn_rl_repo", os.path.expanduser("~/.axon_site/_ro/trn_rl_repo")):
            if os.path.isdir(p) and p not in sys.path:
                sys.path.insert(0, p)
        import concourse  # noqa: F401


def register_pair_op():
    """out = sq(sq(Src0*C0 + C1) + C3): two fused layers, C3 spilled to Src1."""
    _import_concourse()
    from concourse import dve_ops as dvo
    from concourse.dve_spec import (
        C0,
        C1,
        C3,
        Spec,
        Src0,
        _has_src1,
        _spill_c3_to_src1,
        lower,
        sq,
    )
    from concourse.dve_uop import DveOpSpec

    name = "SQ_PAIR_ANT"
    for op in dvo.OPS:
        if op.name == name:
            return op

    def _ref(in0, in1, s0, s1, imm2):
        x = in0.astype(np.float32)
        bb = np.asarray(in1, dtype=np.float32).reshape(x.shape[0], -1)[:, :1]
        v = (x * s0 + s1).astype(np.float32)
        o1 = (v * v).astype(np.float32)
        v2 = (o1 + bb).astype(np.float32)
        return (v2 * v2).astype(np.float32)

    body = _spill_c3_to_src1(sq(sq(Src0 * C0 + C1) + C3))
    spec = Spec(body=body, reference=_ref)
    row = max(dvo._SUB_OPCODE_FOR_NAME.values()) + 1
    uops = lower(spec, ver="v3")
    sha = DveOpSpec(name=name, opcode=row, uops=uops, rd1_en=_has_src1(spec)).sha("v3")
    op = dvo.DveOp(name=name, spec=spec, subdim=False, uops_sha={"v3": sha})
    dvo.OPS.append(op)
    dvo._SUB_OPCODE_FOR_NAME[name] = row
    dvo.CUSTOM_DVE_SPECS[name] = spec
    return op


def register_scan_op(name, alu_name, init_name):
    """f32 min/max scan with accum, with an authored 2X_2P perf variant
    (2 elem/cycle via both SBUF read ports; mirrors the stock tensor_scalar
    2X_2P control conventions). out = in (passthrough), accum_out = min/max."""
    import copy as _copy

    _import_concourse()
    from concourse import dve_ops as dvo
    from concourse.dve_spec import Leaf, Spec, Src0, lower
    from concourse.dve_uop import AluInp, AluOp as UAlu, DveOpSpec, InpSel, OutPath, OutSel

    for op in dvo.OPS:
        if op.name == name:
            return op
    alu = getattr(UAlu, alu_name)
    init_sel = getattr(InpSel, init_name)
    spec = Spec(body=Src0, accum=alu, accum_init=Leaf(init_sel))
    uops1x = lower(spec, ver="v3")
    assert len(uops1x) == 2
    seed2p = _copy.deepcopy(uops1x[0])
    st = _copy.deepcopy(uops1x[1])
    st.enable_input(InpSel.SRC_1, 3)  # second stream on lane 2
    st.require_inp1 = 1
    for b in st.datapath_config:
        b.pass_through_delay(2)
    st.datapath_config[0].enable_alu(alu, AluInp.PREV_DELAY_0, AluInp.PREV_DELAY_2)
    st.enable_output(OutSel.DELAY_2, OutPath.WR1_LO)
    uops2p = [seed2p, st]
    row = max(dvo._SUB_OPCODE_FOR_NAME.values()) + 1
    dspec = DveOpSpec(
        name=name,
        opcode=row,
        uops=uops1x,
        uops_2x=uops2p,
        uops_2x_2p=uops2p,
        uops_4x=None,
        perf_max=2,
        rd1_en=False,
    )
    op = dvo.DveOp(name=name, spec=spec, subdim=False, uops_sha={"v3": dspec.sha("v3")})
    dvo.OPS.append(op)
    dvo._SUB_OPCODE_FOR_NAME[name] = row
    dvo.CUSTOM_DVE_SPECS[name] = spec
    dvo._COMPILE_CACHE[(name, "v3")] = dspec
    return op


def build_nc(enable_asserts=False):
    _import_concourse()
    import concourse.bacc as bacc
    import concourse.tile as tile
    from concourse import bass_isa, mybir

    pair_op = register_pair_op()
    scan_min = register_scan_op("SCAN_MIN_2P_ANT", "MIN", "MAX_POS")
    scan_max = register_scan_op("SCAN_MAX_2P_ANT", "MAX", "MAX_NEG")

    f32 = mybir.dt.float32
    Alu = mybir.AluOpType
    Act = mybir.ActivationFunctionType
    AX = mybir.AxisListType

    nc = bacc.Bacc(
        "TRN2",
        target_bir_lowering=False,
        debug=False,
        enable_asserts=enable_asserts,
        num_devices=N_CORES,
    )

    xs = nc.dram_tensor("xs", [CH_PER_CORE, 128, F_FULL], f32, kind="ExternalInput").ap()
    coef = nc.dram_tensor("coef", [128, NCOEF], f32, kind="ExternalInput").ap()
    ys = nc.dram_tensor("ys", [CH_PER_CORE, 128, F_FULL], f32, kind="ExternalOutput").ap()

    with tile.TileContext(nc) as tc:
        with (
            tc.tile_pool(name="data", bufs=3) as dpool,
            tc.tile_pool(name="cst", bufs=1) as cpool,
            tc.tile_pool(name="st", bufs=2) as st,
            tc.tile_pool(name="pt", bufs=4) as pt,
        ):
            coeft = cpool.tile([128, NCOEF], f32, tag="coeft", name="coeft")
            nc.sync.dma_start(out=coeft[:], in_=coef)

            def cv(nm, l):
                base = _COEF_NAMES.index(nm) * _NL4 + l * CH_PER_CORE
                return coeft[:, base : base + CH_PER_CORE]

            cf0v = coeft[:, len(_COEF_NAMES) * _NL4 :]

            alphaT = cpool.tile([128, N_PAIRS * 4], f32, tag="alphaT", name="alphaT")
            betaT = cpool.tile([128, N_PAIRS * 4], f32, tag="betaT", name="betaT")
            betabT = cpool.tile([128, N_PAIRS * 4], f32, tag="betabT", name="betabT")
            cf1T = cpool.tile([128, 4], f32, tag="cf1T", name="cf1T")
            mn0t = cpool.tile([128, 4], f32, tag="mn0t", name="mn0t")
            mx0t = cpool.tile([128, 4], f32, tag="mx0t", name="mx0t")

            def s4(tag):
                return st.tile([128, 4], f32, tag=tag, name=tag)

            # ---------- Phase 1: DMA in + layer-0 min/max scans ----------
            def scan_chunk(src_chunk, c, k, pmn, pmx):
                i1 = nc.vector._custom_dve(
                    scan_min, out=src_chunk, in0=src_chunk, accum_out=pmn[:, k : k + 1]
                )
                i1.perf_max = 2
                i2 = nc.vector._custom_dve(
                    scan_max, out=src_chunk, in0=src_chunk, accum_out=pmx[:, k : k + 1]
                )
                i2.perf_max = 2

            def combine(c, pmn, pmx):
                rmn = pt.tile([128, 1], f32, tag="rmn", name="rmn")
                rmx = pt.tile([128, 1], f32, tag="rmx", name="rmx")
                nc.vector.tensor_reduce(rmn[:], pmn[:], axis=AX.X, op=Alu.min)
                nc.vector.tensor_reduce(rmx[:], pmx[:], axis=AX.X, op=Alu.max)
                nc.vector.tensor_scalar_mul(rmn[:], rmn[:], -1.0)
                nmn = pt.tile([128, 1], f32, tag="nmn", name="nmn")
                nc.gpsimd.partition_all_reduce(nmn[:], rmn[:], 128, bass_isa.ReduceOp.max)
                nc.vector.tensor_scalar_mul(mn0t[:, c : c + 1], nmn[:], -1.0)
                nc.gpsimd.partition_all_reduce(
                    mx0t[:, c : c + 1], rmx[:], 128, bass_isa.ReduceOp.max
                )

            # ch3 stream-scanned via two chunk-scratch acquisitions (slots 0,1)
            scr = [
                dpool.tile([128, CW], f32, tag="W", name=f"scr{i}") for i in range(2)
            ]
            pmn3 = pt.tile([128, NCHUNK], f32, tag="pmn", name="pmn3")
            pmx3 = pt.tile([128, NCHUNK], f32, tag="pmx", name="pmx3")
            for k in range(NCHUNK):
                s = scr[k % 2]
                nc.sync.dma_start(out=s[:], in_=xs[3][:, k * CW : (k + 1) * CW])
                scan_chunk(s[:], 3, k, pmn3, pmx3)

            W = {}
            for c in range(3):
                W[c] = dpool.tile([128, F_FULL], f32, tag="W", name=f"W{c}")
                pmn = pt.tile([128, NCHUNK], f32, tag="pmn", name=f"pmn{c}")
                pmx = pt.tile([128, NCHUNK], f32, tag="pmx", name=f"pmx{c}")
                for k in range(NCHUNK):
                    ck = W[c][:, k * CW : (k + 1) * CW]
                    nc.sync.dma_start(out=ck, in_=xs[c][:, k * CW : (k + 1) * CW])
                    scan_chunk(ck, c, k, pmn, pmx)
                combine(c, pmn, pmx)
            combine(3, pmn3, pmx3)

            # ---------- Phase 2+3 interleaved: chain (1 pair lookahead) + units
            # boot
            D0 = s4("D0")
            nc.vector.tensor_sub(D0[:], mx0t[:], mn0t[:])
            Dse0 = s4("Dse0")
            nc.vector.tensor_scalar_add(Dse0[:], D0[:], EPS)
            sp0 = s4("sp0")
            nc.vector.reciprocal(sp0[:], Dse0[:])
            A = s4("A")
            nc.vector.tensor_scalar(A[:], sp0[:], -EPS, 1.0, Alu.mult, Alu.add)
            rgamma = sp0
            delta = mn0t

            state = {"A": A, "rgamma": rgamma, "delta": delta, "gmid": None, "dmid": None}

            def chain_layer_stats(l):
                t1 = s4("t1")
                nc.vector.tensor_add(t1[:], state["A"][:], cv("d2", l))
                eA = s4("eA")
                nc.vector.tensor_mul(eA[:], t1[:], t1[:])
                i_ = s4("i_")
                nc.vector.scalar_tensor_tensor(
                    i_[:], t1[:], 0.0, cv("nf", l), Alu.is_gt, Alu.mult
                )
                j = s4("j")
                nc.vector.tensor_scalar(j[:], i_[:], -1.0, 1.0, Alu.mult, Alu.add)
                mne = s4("mne")
                nc.vector.tensor_tensor(mne[:], eA[:], cv("e0", l), Alu.min)
                mn = s4("mn")
                nc.vector.tensor_mul(mn[:], mne[:], j[:])
                mx = s4("mx")
                nc.vector.tensor_tensor(mx[:], eA[:], cv("e0", l), Alu.max)
                spr = s4("spr")
                nc.vector.tensor_sub(spr[:], mx[:], mn[:])
                tg = s4("tg")
                nc.vector.tensor_mul(tg[:], spr[:], cv("g", l))
                E = s4("E")
                nc.vector.tensor_sub(E[:], mx[:], tg[:])
                tD = s4("tD")
                nc.vector.tensor_mul(tD[:], spr[:], cv("absa2", l))
                Dse = s4("Dse")
                nc.vector.tensor_scalar_add(Dse[:], tD[:], EPS)
                sp = s4("sp")
                nc.vector.reciprocal(sp[:], Dse[:])
                Anew = s4("Anew")
                nc.vector.tensor_scalar(Anew[:], sp[:], -EPS, 1.0, Alu.mult, Alu.add)
                state["A"] = Anew
                return E, Dse, sp

            def chain_pair(p):
                lA, lB = 2 * p, 2 * p + 1
                av = alphaT[:, p * 4 : p * 4 + 4]
                bv = betaT[:, p * 4 : p * 4 + 4]
                bbv = betabT[:, p * 4 : p * 4 + 4]
                # A-layer
                E, Dse, sp = chain_layer_stats(lA)
                aspa = s4("aspa")
                nc.vector.tensor_mul(aspa[:], cv("absa2", lA), sp[:])
                w = s4("w")
                nc.scalar.activation(w[:], aspa[:], Act.Sqrt)
                w2 = s4("w2")
                nc.vector.tensor_mul(w2[:], w[:], w[:])
                raspa = s4("raspa")
                nc.vector.tensor_mul(raspa[:], Dse[:], cv("r_absa2", lA))
                gmu = s4("gmu")
                nc.vector.tensor_mul(gmu[:], w2[:], raspa[:])
                gmid = s4("gmid")
                nc.vector.tensor_mul(gmid[:], gmu[:], cv("sgn", lA))
                nc.vector.tensor_mul(av, w[:], state["rgamma"][:])
                tad = s4("tad")
                nc.vector.tensor_mul(tad[:], av, state["delta"][:])
                twd = s4("twd")
                nc.vector.tensor_mul(twd[:], w[:], cv("d2", lA))
                nc.vector.tensor_sub(bv, twd[:], tad[:])
                dmid = s4("dmid")
                nc.vector.tensor_mul(dmid[:], w2[:], E[:])
                # B-layer
                E2, Dse2, _sp2 = chain_layer_stats(lB)
                tbd = s4("tbd")
                nc.vector.tensor_mul(tbd[:], gmid[:], cv("d2", lB))
                nc.vector.tensor_sub(bbv, tbd[:], dmid[:])
                gm2 = s4("gm2")
                nc.vector.tensor_mul(gm2[:], gmid[:], gmid[:])
                if p < N_PAIRS - 1:
                    tg2 = s4("tg2")
                    nc.vector.tensor_mul(tg2[:], gm2[:], Dse2[:])
                    gam = s4("gam")
                    nc.vector.tensor_mul(gam[:], tg2[:], cv("r_a2", lB))
                    dele = s4("dele")
                    nc.vector.tensor_mul(dele[:], gm2[:], E2[:])
                    rg = s4("rg")
                    nc.vector.reciprocal(rg[:], gam[:])
                    state["rgamma"] = rg
                    state["delta"] = dele
                else:
                    rgm2 = s4("rgm2")
                    nc.vector.reciprocal(rgm2[:], gm2[:])
                    # cf1 = a2cl / gm2 ; a2cl = absa2*sgn
                    a2c = s4("a2c")
                    nc.vector.tensor_mul(a2c[:], cv("absa2", lB), cv("sgn", lB))
                    nc.vector.tensor_mul(cf1T[:], a2c[:], rgm2[:])

            def unit(c, k, p):
                ck = W[c][:, k * CW : (k + 1) * CW]
                a_ap = alphaT[:, p * 4 + c : p * 4 + c + 1]
                b_ap = betaT[:, p * 4 + c : p * 4 + c + 1]
                bb_ap = betabT[:, p * 4 + c : p * 4 + c + 1]
                if unit_engine_is_act(c, k, p):
                    nc.scalar.activation(ck, ck, Act.Square, bias=b_ap, scale=a_ap)
                    nc.scalar.activation(ck, ck, Act.Square, bias=bb_ap, scale=1.0)
                else:
                    nc.vector._custom_dve(
                        pair_op, out=ck, in0=ck, in1=bb_ap, s0=a_ap, s1=b_ap
                    )

            def finish_chunk(c, k):
                ck = W[c][:, k * CW : (k + 1) * CW]
                cf1_ap = cf1T[:, c : c + 1]
                cf0_ap = cf0v[:, c : c + 1]
                if affine_engine_is_act(c, k):
                    nc.scalar.activation(ck, ck, Act.Identity, bias=cf0_ap, scale=cf1_ap)
                else:
                    nc.vector.tensor_scalar(ck, ck, cf1_ap, cf0_ap, Alu.mult, Alu.add)
                nc.sync.dma_start(out=ys[c][:, k * CW : (k + 1) * CW], in_=ck)

            # chain runs 1 pair ahead of the units of channels 0-2
            chain_pair(0)
            for p in range(N_PAIRS):
                if p + 1 < N_PAIRS:
                    chain_pair(p + 1)
                for c in range(3):
                    for k in range(NCHUNK):
                        unit(c, k, p)
            for c in range(3):
                for k in range(NCHUNK):
                    finish_chunk(c, k)

            # ---------- Phase 3b: channel 3 (buffer freed by ch0) ----------
            W[3] = dpool.tile([128, F_FULL], f32, tag="W", name="W3")
            for k in range(NCHUNK):
                nc.sync.dma_start(
                    out=W[3][:, k * CW : (k + 1) * CW],
                    in_=xs[3][:, k * CW : (k + 1) * CW],
                )
            for k in range(NCHUNK):
                for p in range(N_PAIRS):
                    unit(3, k, p)
                finish_chunk(3, k)

    nc.compile()
    return nc


_NC_CACHE = {}


def _get_nc():
    if "full" not in _NC_CACHE:
        _NC_CACHE["full"] = build_nc()
    return _NC_CACHE["full"]


def host_coefs(w0, w1, w2):
    """Per-core coef arrays [128, NCOEF] (f32, broadcast over partitions)."""
    f = np.float32
    a2 = np.asarray(w2, dtype=f)
    a1 = np.asarray(w1, dtype=f)
    a0 = np.asarray(w0, dtype=f)
    sgn = np.where(a2 >= 0, f(1), f(-1)).astype(f)
    a2cl = (sgn * np.maximum(np.abs(a2), f(CLAMP))).astype(f)
    d2 = (a1 / a2cl / 2).astype(f)
    e0 = (d2 * d2).astype(f)
    nf = (d2 < 0).astype(f)
    g = (a2cl >= 0).astype(f)
    absa2 = np.abs(a2cl).astype(f)
    r_a2 = (f(1) / a2cl).astype(f)
    r_absa2 = (f(1) / absa2).astype(f)
    arrays = {
        "d2": d2, "e0": e0, "nf": nf, "g": g,
        "absa2": absa2, "r_a2": r_a2, "sgn": sgn, "r_absa2": r_absa2,
    }
    cf0 = (a0[N_LAYERS - 1] - a2cl[N_LAYERS - 1] * e0[N_LAYERS - 1]).astype(f)

    out = []
    for core in range(N_CORES):
        cols = slice(CH_PER_CORE * core, CH_PER_CORE * (core + 1))
        row = np.empty(NCOEF, dtype=f)
        for idx, nm in enumerate(_COEF_NAMES):
            arr = arrays[nm][:, cols]  # [NL, 4]
            row[idx * _NL4 : (idx + 1) * _NL4] = arr.reshape(-1)  # l*4+c
        row[len(_COEF_NAMES) * _NL4 :] = cf0[cols]
        out.append(np.ascontiguousarray(np.broadcast_to(row[None, :], (128, NCOEF))))
    return out


def shard_inputs(x, w0, w1, w2):
    x = np.ascontiguousarray(x, dtype=np.float32)
    coefs = host_coefs(w0, w1, w2)
    in_maps = []
    for k in range(N_CORES):
        cols = slice(CH_PER_CORE * k, CH_PER_CORE * (k + 1))
        xk = np.ascontiguousarray(x[:, cols].transpose(1, 0, 2, 3)).reshape(
            CH_PER_CORE, 128, F_FULL
        )
        in_maps.append({"xs": xk, "coef": coefs[k]})
    return in_maps


def unshard_output(results):
    out = np.empty((B, C, H, Wd), dtype=np.float32)
    for k in range(N_CORES):
        ysk = np.asarray(results[k]["ys"], dtype=np.float32).reshape(
            CH_PER_CORE, B, H, Wd
        )
        out[:, CH_PER_CORE * k : CH_PER_CORE * (k + 1)] = ysk.transpose(1, 0, 2, 3)
    return out


def run_sharded(in_maps, trace=False, trace_kwargs=None):
    _import_concourse()
    from concourse.bass_utils import run_bass_kernel_spmd

    nc = _get_nc()
    return run_bass_kernel_spmd(
        nc,
        in_maps,
        core_ids=list(range(N_CORES)),
        trace=trace,
        **(trace_kwargs or {}),
    )


def kernel(x, w0, w1, w2):
    in_maps = shard_inputs(x, w0, w1, w2)
    res = run_sharded(in_maps)
    return unshard_output(res.results)


# revision 10
# speedup vs baseline: 1.1325x; 1.1181x over previous
"""Trainium2 Bass kernel for nn_ActSeries: 20 layers of per-channel range-norm +
quadratic polynomial, x [32,32,256,256] f32.

Strategy (v2 — analytic range propagation, dual-engine streaming)
-----------------------------------------------------------------
Shard the 32 channels across 8 cores (4 channels/core); per-channel stats make
every reduction core-local (no collectives).

Math: each layer is h' = a2*xh^2 + a1*xh + a0 with xh = (h-mn)/(mx-mn+eps).
Complete the square: h' = a2*(xh + d2)^2 + const, d2 = a1/(2*a2). The range-norm
is invariant to tracked affine maps, so we store Z = gamma*xh + delta and fold
each layer into Z' = (alpha*Z + beta)^2 (one multiply-add-square per element).
Key observation: the data min/max of the NEXT layer is analytic given this
layer's range [0, A]: max over the interval is attained at an endpoint (both
endpoints ARE data points), and the interior-vertex min is ~0 to within the
data spacing squared (~1e-12), far below the 2e-2 tolerance. So after a single
min/max scan of the raw input (layer 0), all 20 layers' scale/offset constants
follow from a tiny per-channel scalar recurrence — no more data scans, no
inter-layer dependencies beyond the elementwise stream.

Per-pair affine normalization: the A-layer (even) picks its output scale
w = sqrt(|a2*s'|) so gamma_mid = +-1; the B-layer (odd) then needs no scale:
Z'' = (Z' + betab)^2. Two layers fuse into ONE 5-stage custom DVE op
  out = sq(sq(Src0*C0 + C1) + C3)   (C0=alpha, C1=beta, C3=betab via Src1 latch)
at 1 elem/cycle, i.e. 2 layer-elements/cycle. The Scalar engine computes the
same layers via ACTIVATE Square ((scale*x+bias)^2), so DVE and ACT split the
chunks ~5:3 and run concurrently. Final y = cf1*Z + cf0 in one affine pass.
Everything runs in place (verified on HW); 3 channel buffers rotate in SBUF.

Validated end-to-end in numpy against the reference: rel err ~2e-4.
"""

import os
import sys

import numpy as np

B, C, H, Wd = 32, 32, 256, 256
N_LAYERS = 20
N_PAIRS = N_LAYERS // 2
EPS = 1e-5
N_CORES = 8
CH_PER_CORE = C // N_CORES  # 4
F_FULL = B * H * Wd // 128  # 16384 free-dim elements per partition
CW = 4096
NCHUNK = F_FULL // CW  # 4
CLAMP = 1e-4  # |a2| clamp; error bounded by CLAMP*A^2 << tol

# Engine assignment: channels 0-2 chunk-static (ACT takes chunks {1,3});
# channel 3 (the deferred-buffer channel) is pair-level mixed so both engines
# share its tail. Ratio tuned for DVE pair-op 4.54us vs ACT 2-activate 7.0us.
def unit_engine_is_act(c, k, p):
    return (c, k) in {(0, 2), (1, 1), (1, 3), (2, 2), (3, 1), (3, 3)}


def affine_engine_is_act(c, k):
    return unit_engine_is_act(c, k, 0)


def emit_scan(nc, op, out, in0, accum_out):
    """Emit a scan custom-DVE op with perf_max=2 so the engine may select the
    authored 2X_2P program (perf_max must be set at construction — the 64B
    instruction image is encoded eagerly)."""
    _import_concourse()
    from concourse import bass_isa, mybir
    from concourse.dve_ops import get_dve_sub_opcode

    vec = nc.vector
    if op.name not in vec.bass.m.ant_custom_dve_ops:
        vec.bass.m.ant_custom_dve_ops = sorted(
            {*vec.bass.m.ant_custom_dve_ops, op.name}
        )
    op.compile("v3")
    shape = bass_isa.CustomDveShape.TTSS
    isa_opcode = vec.bass.isa.Opcode[
        f"NEURON_ISA_TPB_OPCODE_CUSTOM_DVE_ANT_{shape.slot()}"
    ].value
    imm0 = mybir.ImmediateValue(dtype=mybir.dt.float32, value=0.0)
    ins = [vec.lower_ap(in0, for_isa=True, opt=True), imm0, imm0]
    outs = [
        vec.lower_ap(out, for_isa=True, opt=True),
        vec.lower_ap(accum_out, for_isa=True),
    ]
    return vec.add_instruction(
        bass_isa.InstCustomDveAnt(
            name=vec.bass.get_next_instruction_name(),
            op_name=op.name,
            rd1_en=False,
            subdim=0,
            imm2=0.0,
            shape=shape,
            row=get_dve_sub_opcode(op.name),
            isa_opcode=isa_opcode,
            ins=ins,
            outs=outs,
            perf_max=2,
        )
    )

# coef column layout: 8 per-layer arrays of [N_LAYERS*4] (l*4+c), then cf0 [4]
_NL4 = N_LAYERS * CH_PER_CORE  # 80
_COEF_NAMES = ("d2", "e0", "nf", "g", "absa2", "r_a2", "sgn", "r_absa2")
NCOEF = len(_COEF_NAMES) * _NL4 + CH_PER_CORE  # 644


def _import_concourse():
    try:
        import concourse  # noqa: F401
    except ImportError:
        for p in ("/opt/trn_rl_repo", os.path.expanduser("~/.axon_site/_ro/trn_rl_repo")):
            if os.path.isdir(p) and p not in sys.path:
                sys.path.insert(0, p)
        import concourse  # noqa: F401


def register_pair_op():
    """out = sq(sq(Src0*C0 + C1) + C3): two fused layers, C3 spilled to Src1."""
    _import_concourse()
    from concourse import dve_ops as dvo
    from concourse.dve_spec import (
        C0,
        C1,
        C3,
        Spec,
        Src0,
        _has_src1,
        _spill_c3_to_src1,
        lower,
        sq,
    )
    from concourse.dve_uop import DveOpSpec

    name = "SQ_PAIR_ANT"
    for op in dvo.OPS:
        if op.name == name:
            return op

    def _ref(in0, in1, s0, s1, imm2):
        x = in0.astype(np.float32)
        bb = np.asarray(in1, dtype=np.float32).reshape(x.shape[0], -1)[:, :1]
        v = (x * s0 + s1).astype(np.float32)
        o1 = (v * v).astype(np.float32)
        v2 = (o1 + bb).astype(np.float32)
        return (v2 * v2).astype(np.float32)

    body = _spill_c3_to_src1(sq(sq(Src0 * C0 + C1) + C3))
    spec = Spec(body=body, reference=_ref)
    row = max(dvo._SUB_OPCODE_FOR_NAME.values()) + 1
    uops = lower(spec, ver="v3")
    sha = DveOpSpec(name=name, opcode=row, uops=uops, rd1_en=_has_src1(spec)).sha("v3")
    op = dvo.DveOp(name=name, spec=spec, subdim=False, uops_sha={"v3": sha})
    dvo.OPS.append(op)
    dvo._SUB_OPCODE_FOR_NAME[name] = row
    dvo.CUSTOM_DVE_SPECS[name] = spec
    return op


def register_scan_op(name, alu_name, init_name):
    """f32 min/max scan with accum, with an authored 2X_2P perf variant
    (2 elem/cycle via both SBUF read ports; mirrors the stock tensor_scalar
    2X_2P control conventions). out = in (passthrough), accum_out = min/max."""
    import copy as _copy

    _import_concourse()
    from concourse import dve_ops as dvo
    from concourse.dve_spec import Leaf, Spec, Src0, lower
    from concourse.dve_uop import AluInp, AluOp as UAlu, DveOpSpec, InpSel, OutPath, OutSel

    for op in dvo.OPS:
        if op.name == name:
            return op
    alu = getattr(UAlu, alu_name)
    init_sel = getattr(InpSel, init_name)
    spec = Spec(body=Src0, accum=alu, accum_init=Leaf(init_sel))
    uops1x = lower(spec, ver="v3")
    assert len(uops1x) == 2
    seed2p = _copy.deepcopy(uops1x[0])
    st = _copy.deepcopy(uops1x[1])
    st.enable_input(InpSel.SRC_1, 3)  # second stream on lane 2
    st.require_inp1 = 1
    for b in st.datapath_config:
        b.pass_through_delay(2)
    st.datapath_config[0].enable_alu(alu, AluInp.PREV_DELAY_0, AluInp.PREV_DELAY_2)
    st.enable_output(OutSel.DELAY_2, OutPath.WR1_LO)
    uops2p = [seed2p, st]
    row = max(dvo._SUB_OPCODE_FOR_NAME.values()) + 1
    dspec = DveOpSpec(
        name=name,
        opcode=row,
        uops=uops1x,
        uops_2x=uops2p,
        uops_2x_2p=uops2p,
        uops_4x=None,
        perf_max=2,
        rd1_en=False,
    )
    op = dvo.DveOp(name=name, spec=spec, subdim=False, uops_sha={"v3": dspec.sha("v3")})
    dvo.OPS.append(op)
    dvo._SUB_OPCODE_FOR_NAME[name] = row
    dvo.CUSTOM_DVE_SPECS[name] = spec
    dvo._COMPILE_CACHE[(name, "v3")] = dspec
    return op


def build_nc(enable_asserts=False):
    _import_concourse()
    import concourse.bacc as bacc
    import concourse.tile as tile
    from concourse import bass_isa, mybir

    pair_op = register_pair_op()
    scan_min = register_scan_op("SCAN_MIN_2P_ANT", "MIN", "MAX_POS")
    scan_max = register_scan_op("SCAN_MAX_2P_ANT", "MAX", "MAX_NEG")

    f32 = mybir.dt.float32
    Alu = mybir.AluOpType
    Act = mybir.ActivationFunctionType
    AX = mybir.AxisListType

    nc = bacc.Bacc(
        "TRN2",
        target_bir_lowering=False,
        debug=False,
        enable_asserts=enable_asserts,
        num_devices=N_CORES,
    )

    xs = nc.dram_tensor("xs", [CH_PER_CORE, 128, F_FULL], f32, kind="ExternalInput").ap()
    coef = nc.dram_tensor("coef", [128, NCOEF], f32, kind="ExternalInput").ap()
    ys = nc.dram_tensor("ys", [CH_PER_CORE, 128, F_FULL], f32, kind="ExternalOutput").ap()

    with tile.TileContext(nc) as tc:
        with (
            tc.tile_pool(name="data", bufs=3) as dpool,
            tc.tile_pool(name="cst", bufs=1) as cpool,
            tc.tile_pool(name="st", bufs=2) as st,
            tc.tile_pool(name="pt", bufs=4) as pt,
        ):
            coeft = cpool.tile([128, NCOEF], f32, tag="coeft", name="coeft")
            nc.sync.dma_start(out=coeft[:], in_=coef)

            def cv(nm, l):
                base = _COEF_NAMES.index(nm) * _NL4 + l * CH_PER_CORE
                return coeft[:, base : base + CH_PER_CORE]

            cf0v = coeft[:, len(_COEF_NAMES) * _NL4 :]

            alphaT = cpool.tile([128, N_PAIRS * 4], f32, tag="alphaT", name="alphaT")
            betaT = cpool.tile([128, N_PAIRS * 4], f32, tag="betaT", name="betaT")
            betabT = cpool.tile([128, N_PAIRS * 4], f32, tag="betabT", name="betabT")
            cf1T = cpool.tile([128, 4], f32, tag="cf1T", name="cf1T")
            mn0t = cpool.tile([128, 4], f32, tag="mn0t", name="mn0t")
            mx0t = cpool.tile([128, 4], f32, tag="mx0t", name="mx0t")

            def s4(tag):
                return st.tile([128, 4], f32, tag=tag, name=tag)

            # ---------- Phase 1: DMA in + layer-0 min/max scans ----------
            def scan_chunk(src_chunk, c, k, pmn, pmx):
                emit_scan(nc, scan_min, src_chunk, src_chunk, pmn[:, k : k + 1])
                emit_scan(nc, scan_max, src_chunk, src_chunk, pmx[:, k : k + 1])

            def combine(c, pmn, pmx):
                rmn = pt.tile([128, 1], f32, tag="rmn", name="rmn")
                rmx = pt.tile([128, 1], f32, tag="rmx", name="rmx")
                nc.vector.tensor_reduce(rmn[:], pmn[:], axis=AX.X, op=Alu.min)
                nc.vector.tensor_reduce(rmx[:], pmx[:], axis=AX.X, op=Alu.max)
                nc.vector.tensor_scalar_mul(rmn[:], rmn[:], -1.0)
                nmn = pt.tile([128, 1], f32, tag="nmn", name="nmn")
                nc.gpsimd.partition_all_reduce(nmn[:], rmn[:], 128, bass_isa.ReduceOp.max)
                nc.vector.tensor_scalar_mul(mn0t[:, c : c + 1], nmn[:], -1.0)
                nc.gpsimd.partition_all_reduce(
                    mx0t[:, c : c + 1], rmx[:], 128, bass_isa.ReduceOp.max
                )

            # ch3 stream-scanned via two chunk-scratch acquisitions (slots 0,1)
            scr = [
                dpool.tile([128, CW], f32, tag="W", name=f"scr{i}") for i in range(2)
            ]
            pmn3 = pt.tile([128, NCHUNK], f32, tag="pmn", name="pmn3")
            pmx3 = pt.tile([128, NCHUNK], f32, tag="pmx", name="pmx3")
            for k in range(NCHUNK):
                s = scr[k % 2]
                nc.sync.dma_start(out=s[:], in_=xs[3][:, k * CW : (k + 1) * CW])
                scan_chunk(s[:], 3, k, pmn3, pmx3)

            W = {}
            for c in range(3):
                W[c] = dpool.tile([128, F_FULL], f32, tag="W", name=f"W{c}")
                pmn = pt.tile([128, NCHUNK], f32, tag="pmn", name=f"pmn{c}")
                pmx = pt.tile([128, NCHUNK], f32, tag="pmx", name=f"pmx{c}")
                for k in range(NCHUNK):
                    ck = W[c][:, k * CW : (k + 1) * CW]
                    nc.sync.dma_start(out=ck, in_=xs[c][:, k * CW : (k + 1) * CW])
                    scan_chunk(ck, c, k, pmn, pmx)
                combine(c, pmn, pmx)
            combine(3, pmn3, pmx3)

            # ---------- Phase 2+3 interleaved: chain (1 pair lookahead) + units
            # boot
            D0 = s4("D0")
            nc.vector.tensor_sub(D0[:], mx0t[:], mn0t[:])
            Dse0 = s4("Dse0")
            nc.vector.tensor_scalar_add(Dse0[:], D0[:], EPS)
            sp0 = s4("sp0")
            nc.vector.reciprocal(sp0[:], Dse0[:])
            A = s4("A")
            nc.vector.tensor_scalar(A[:], sp0[:], -EPS, 1.0, Alu.mult, Alu.add)
            rgamma = sp0
            delta = mn0t

            state = {"A": A, "rgamma": rgamma, "delta": delta, "gmid": None, "dmid": None}

            def chain_layer_stats(l):
                t1 = s4("t1")
                nc.vector.tensor_add(t1[:], state["A"][:], cv("d2", l))
                eA = s4("eA")
                nc.vector.tensor_mul(eA[:], t1[:], t1[:])
                i_ = s4("i_")
                nc.vector.scalar_tensor_tensor(
                    i_[:], t1[:], 0.0, cv("nf", l), Alu.is_gt, Alu.mult
                )
                j = s4("j")
                nc.vector.tensor_scalar(j[:], i_[:], -1.0, 1.0, Alu.mult, Alu.add)
                mne = s4("mne")
                nc.vector.tensor_tensor(mne[:], eA[:], cv("e0", l), Alu.min)
                mn = s4("mn")
                nc.vector.tensor_mul(mn[:], mne[:], j[:])
                mx = s4("mx")
                nc.vector.tensor_tensor(mx[:], eA[:], cv("e0", l), Alu.max)
                spr = s4("spr")
                nc.vector.tensor_sub(spr[:], mx[:], mn[:])
                tg = s4("tg")
                nc.vector.tensor_mul(tg[:], spr[:], cv("g", l))
                E = s4("E")
                nc.vector.tensor_sub(E[:], mx[:], tg[:])
                tD = s4("tD")
                nc.vector.tensor_mul(tD[:], spr[:], cv("absa2", l))
                Dse = s4("Dse")
                nc.vector.tensor_scalar_add(Dse[:], tD[:], EPS)
                sp = s4("sp")
                nc.vector.reciprocal(sp[:], Dse[:])
                Anew = s4("Anew")
                nc.vector.tensor_scalar(Anew[:], sp[:], -EPS, 1.0, Alu.mult, Alu.add)
                state["A"] = Anew
                return E, Dse, sp

            def chain_pair(p):
                lA, lB = 2 * p, 2 * p + 1
                av = alphaT[:, p * 4 : p * 4 + 4]
                bv = betaT[:, p * 4 : p * 4 + 4]
                bbv = betabT[:, p * 4 : p * 4 + 4]
                # A-layer
                E, Dse, sp = chain_layer_stats(lA)
                aspa = s4("aspa")
                nc.vector.tensor_mul(aspa[:], cv("absa2", lA), sp[:])
                w = s4("w")
                nc.scalar.activation(w[:], aspa[:], Act.Sqrt)
                w2 = s4("w2")
                nc.vector.tensor_mul(w2[:], w[:], w[:])
                raspa = s4("raspa")
                nc.vector.tensor_mul(raspa[:], Dse[:], cv("r_absa2", lA))
                gmu = s4("gmu")
                nc.vector.tensor_mul(gmu[:], w2[:], raspa[:])
                gmid = s4("gmid")
                nc.vector.tensor_mul(gmid[:], gmu[:], cv("sgn", lA))
                nc.vector.tensor_mul(av, w[:], state["rgamma"][:])
                tad = s4("tad")
                nc.vector.tensor_mul(tad[:], av, state["delta"][:])
                twd = s4("twd")
                nc.vector.tensor_mul(twd[:], w[:], cv("d2", lA))
                nc.vector.tensor_sub(bv, twd[:], tad[:])
                dmid = s4("dmid")
                nc.vector.tensor_mul(dmid[:], w2[:], E[:])
                # B-layer
                E2, Dse2, _sp2 = chain_layer_stats(lB)
                tbd = s4("tbd")
                nc.vector.tensor_mul(tbd[:], gmid[:], cv("d2", lB))
                nc.vector.tensor_sub(bbv, tbd[:], dmid[:])
                gm2 = s4("gm2")
                nc.vector.tensor_mul(gm2[:], gmid[:], gmid[:])
                if p < N_PAIRS - 1:
                    tg2 = s4("tg2")
                    nc.vector.tensor_mul(tg2[:], gm2[:], Dse2[:])
                    gam = s4("gam")
                    nc.vector.tensor_mul(gam[:], tg2[:], cv("r_a2", lB))
                    dele = s4("dele")
                    nc.vector.tensor_mul(dele[:], gm2[:], E2[:])
                    rg = s4("rg")
                    nc.vector.reciprocal(rg[:], gam[:])
                    state["rgamma"] = rg
                    state["delta"] = dele
                else:
                    rgm2 = s4("rgm2")
                    nc.vector.reciprocal(rgm2[:], gm2[:])
                    # cf1 = a2cl / gm2 ; a2cl = absa2*sgn
                    a2c = s4("a2c")
                    nc.vector.tensor_mul(a2c[:], cv("absa2", lB), cv("sgn", lB))
                    nc.vector.tensor_mul(cf1T[:], a2c[:], rgm2[:])

            def unit(c, k, p):
                ck = W[c][:, k * CW : (k + 1) * CW]
                a_ap = alphaT[:, p * 4 + c : p * 4 + c + 1]
                b_ap = betaT[:, p * 4 + c : p * 4 + c + 1]
                bb_ap = betabT[:, p * 4 + c : p * 4 + c + 1]
                if unit_engine_is_act(c, k, p):
                    nc.scalar.activation(ck, ck, Act.Square, bias=b_ap, scale=a_ap)
                    nc.scalar.activation(ck, ck, Act.Square, bias=bb_ap, scale=1.0)
                else:
                    nc.vector._custom_dve(
                        pair_op, out=ck, in0=ck, in1=bb_ap, s0=a_ap, s1=b_ap
                    )

            def finish_chunk(c, k):
                ck = W[c][:, k * CW : (k + 1) * CW]
                cf1_ap = cf1T[:, c : c + 1]
                cf0_ap = cf0v[:, c : c + 1]
                if affine_engine_is_act(c, k):
                    nc.scalar.activation(ck, ck, Act.Identity, bias=cf0_ap, scale=cf1_ap)
                else:
                    nc.vector.tensor_scalar(ck, ck, cf1_ap, cf0_ap, Alu.mult, Alu.add)
                nc.sync.dma_start(out=ys[c][:, k * CW : (k + 1) * CW], in_=ck)

            # chain runs 1 pair ahead of the units of channels 0-2
            chain_pair(0)
            for p in range(N_PAIRS):
                if p + 1 < N_PAIRS:
                    chain_pair(p + 1)
                for c in range(3):
                    for k in range(NCHUNK):
                        unit(c, k, p)
            for c in range(3):
                for k in range(NCHUNK):
                    finish_chunk(c, k)

            # ---------- Phase 3b: channel 3 (buffer freed by ch0) ----------
            W[3] = dpool.tile([128, F_FULL], f32, tag="W", name="W3")
            for k in range(NCHUNK):
                nc.sync.dma_start(
                    out=W[3][:, k * CW : (k + 1) * CW],
                    in_=xs[3][:, k * CW : (k + 1) * CW],
                )
            for k in range(NCHUNK):
                for p in range(N_PAIRS):
                    unit(3, k, p)
                finish_chunk(3, k)

    nc.compile()
    return nc


_NC_CACHE = {}


def _get_nc():
    if "full" not in _NC_CACHE:
        _NC_CACHE["full"] = build_nc()
    return _NC_CACHE["full"]


def host_coefs(w0, w1, w2):
    """Per-core coef arrays [128, NCOEF] (f32, broadcast over partitions)."""
    f = np.float32
    a2 = np.asarray(w2, dtype=f)
    a1 = np.asarray(w1, dtype=f)
    a0 = np.asarray(w0, dtype=f)
    sgn = np.where(a2 >= 0, f(1), f(-1)).astype(f)
    a2cl = (sgn * np.maximum(np.abs(a2), f(CLAMP))).astype(f)
    d2 = (a1 / a2cl / 2).astype(f)
    e0 = (d2 * d2).astype(f)
    nf = (d2 < 0).astype(f)
    g = (a2cl >= 0).astype(f)
    absa2 = np.abs(a2cl).astype(f)
    r_a2 = (f(1) / a2cl).astype(f)
    r_absa2 = (f(1) / absa2).astype(f)
    arrays = {
        "d2": d2, "e0": e0, "nf": nf, "g": g,
        "absa2": absa2, "r_a2": r_a2, "sgn": sgn, "r_absa2": r_absa2,
    }
    cf0 = (a0[N_LAYERS - 1] - a2cl[N_LAYERS - 1] * e0[N_LAYERS - 1]).astype(f)

    out = []
    for core in range(N_CORES):
        cols = slice(CH_PER_CORE * core, CH_PER_CORE * (core + 1))
        row = np.empty(NCOEF, dtype=f)
        for idx, nm in enumerate(_COEF_NAMES):
            arr = arrays[nm][:, cols]  # [NL, 4]
            row[idx * _NL4 : (idx + 1) * _NL4] = arr.reshape(-1)  # l*4+c
        row[len(_COEF_NAMES) * _NL4 :] = cf0[cols]
        out.append(np.ascontiguousarray(np.broadcast_to(row[None, :], (128, NCOEF))))
    return out


def shard_inputs(x, w0, w1, w2):
    x = np.ascontiguousarray(x, dtype=np.float32)
    coefs = host_coefs(w0, w1, w2)
    in_maps = []
    for k in range(N_CORES):
        cols = slice(CH_PER_CORE * k, CH_PER_CORE * (k + 1))
        xk = np.ascontiguousarray(x[:, cols].transpose(1, 0, 2, 3)).reshape(
            CH_PER_CORE, 128, F_FULL
        )
        in_maps.append({"xs": xk, "coef": coefs[k]})
    return in_maps


def unshard_output(results):
    out = np.empty((B, C, H, Wd), dtype=np.float32)
    for k in range(N_CORES):
        ysk = np.asarray(results[k]["ys"], dtype=np.float32).reshape(
            CH_PER_CORE, B, H, Wd
        )
        out[:, CH_PER_CORE * k : CH_PER_CORE * (k + 1)] = ysk.transpose(1, 0, 2, 3)
    return out


def run_sharded(in_maps, trace=False, trace_kwargs=None):
    _import_concourse()
    from concourse.bass_utils import run_bass_kernel_spmd

    nc = _get_nc()
    return run_bass_kernel_spmd(
        nc,
        in_maps,
        core_ids=list(range(N_CORES)),
        trace=trace,
        **(trace_kwargs or {}),
    )


def kernel(x, w0, w1, w2):
    in_maps = shard_inputs(x, w0, w1, w2)
    res = run_sharded(in_maps)
    return unshard_output(res.results)


# revision 17
# speedup vs baseline: 1.3729x; 1.2122x over previous
"""Trainium2 Bass kernel for nn_ActSeries: 20 layers of per-channel range-norm +
quadratic polynomial, x [32,32,256,256] f32.

Strategy (v2 — analytic range propagation, dual-engine streaming)
-----------------------------------------------------------------
Shard the 32 channels across 8 cores (4 channels/core); per-channel stats make
every reduction core-local (no collectives).

Math: each layer is h' = a2*xh^2 + a1*xh + a0 with xh = (h-mn)/(mx-mn+eps).
Complete the square: h' = a2*(xh + d2)^2 + const, d2 = a1/(2*a2). The range-norm
is invariant to tracked affine maps, so we store Z = gamma*xh + delta and fold
each layer into Z' = (alpha*Z + beta)^2 (one multiply-add-square per element).
Key observation: the data min/max of the NEXT layer is analytic given this
layer's range [0, A]: max over the interval is attained at an endpoint (both
endpoints ARE data points), and the interior-vertex min is ~0 to within the
data spacing squared (~1e-12), far below the 2e-2 tolerance. So after a single
min/max scan of the raw input (layer 0), all 20 layers' scale/offset constants
follow from a tiny per-channel scalar recurrence — no more data scans, no
inter-layer dependencies beyond the elementwise stream.

Per-pair affine normalization: the A-layer (even) picks its output scale
w = sqrt(|a2*s'|) so gamma_mid = +-1; the B-layer (odd) then needs no scale:
Z'' = (Z' + betab)^2. Two layers fuse into ONE 5-stage custom DVE op
  out = sq(sq(Src0*C0 + C1) + C3)   (C0=alpha, C1=beta, C3=betab via Src1 latch)
at 1 elem/cycle, i.e. 2 layer-elements/cycle. The Scalar engine computes the
same layers via ACTIVATE Square ((scale*x+bias)^2), so DVE and ACT split the
chunks ~5:3 and run concurrently. Final y = cf1*Z + cf0 in one affine pass.
Everything runs in place (verified on HW); 3 channel buffers rotate in SBUF.

Validated end-to-end in numpy against the reference: rel err ~2e-4.
"""

import os
import sys

import numpy as np

B, C, H, Wd = 32, 32, 256, 256
N_LAYERS = 20
N_PAIRS = N_LAYERS // 2
EPS = 1e-5
N_CORES = 8
CH_PER_CORE = C // N_CORES  # 4
F_FULL = B * H * Wd // 128  # 16384 free-dim elements per partition
CW = 4096
NCHUNK = F_FULL // CW  # 4
CLAMP = 1e-4  # |a2| clamp; error bounded by CLAMP*A^2 << tol

# Engine assignment: channels 0-2 chunk-static (ACT takes chunks {1,3});
# channel 3 (the deferred-buffer channel) is pair-level mixed so both engines
# share its tail. Ratio tuned for DVE pair-op 4.54us vs ACT 2-activate 7.0us.
def unit_engine_is_act(c, k, p):
    return (c, k) in {(0, 2), (1, 1), (1, 3), (2, 2), (3, 1)}


def affine_engine_is_act(c, k):
    return unit_engine_is_act(c, k, 0)


def emit_scan(nc, op, out, in0, accum_out):
    """Emit a scan custom-DVE op with perf_max=2 so the engine may select the
    authored 2X_2P program (perf_max must be set at construction — the 64B
    instruction image is encoded eagerly)."""
    _import_concourse()
    from concourse import bass_isa, mybir
    from concourse.dve_ops import get_dve_sub_opcode

    vec = nc.vector
    if op.name not in vec.bass.m.ant_custom_dve_ops:
        vec.bass.m.ant_custom_dve_ops = sorted(
            {*vec.bass.m.ant_custom_dve_ops, op.name}
        )
    op.compile("v3")
    shape = bass_isa.CustomDveShape.TTSS
    isa_opcode = vec.bass.isa.Opcode[
        f"NEURON_ISA_TPB_OPCODE_CUSTOM_DVE_ANT_{shape.slot()}"
    ].value
    imm0 = mybir.ImmediateValue(dtype=mybir.dt.float32, value=0.0)
    ins = [vec.lower_ap(in0, for_isa=True, opt=True), imm0, imm0]
    outs = [
        vec.lower_ap(out, for_isa=True, opt=True),
        vec.lower_ap(accum_out, for_isa=True),
    ]
    return vec.add_instruction(
        bass_isa.InstCustomDveAnt(
            name=vec.bass.get_next_instruction_name(),
            op_name=op.name,
            rd1_en=False,
            subdim=0,
            imm2=0.0,
            shape=shape,
            row=get_dve_sub_opcode(op.name),
            isa_opcode=isa_opcode,
            ins=ins,
            outs=outs,
            perf_max=2,
        )
    )

# coef column layout: 8 per-layer arrays of [N_LAYERS*4] (l*4+c), then cf0 [4]
_NL4 = N_LAYERS * CH_PER_CORE  # 80
_COEF_NAMES = ("d2", "e0", "nf", "g", "absa2", "r_a2", "sgn", "r_absa2")
NCOEF = len(_COEF_NAMES) * _NL4 + CH_PER_CORE  # 644


def _import_concourse():
    try:
        import concourse  # noqa: F401
    except ImportError:
        for p in ("/opt/trn_rl_repo", os.path.expanduser("~/.axon_site/_ro/trn_rl_repo")):
            if os.path.isdir(p) and p not in sys.path:
                sys.path.insert(0, p)
        import concourse  # noqa: F401


def register_pair_op():
    """out = sq(sq(Src0*C0 + C1) + C3): two fused layers, C3 spilled to Src1."""
    _import_concourse()
    from concourse import dve_ops as dvo
    from concourse.dve_spec import (
        C0,
        C1,
        C3,
        Spec,
        Src0,
        _has_src1,
        _spill_c3_to_src1,
        lower,
        sq,
    )
    from concourse.dve_uop import DveOpSpec

    name = "SQ_PAIR_ANT"
    for op in dvo.OPS:
        if op.name == name:
            return op

    def _ref(in0, in1, s0, s1, imm2):
        x = in0.astype(np.float32)
        bb = np.asarray(in1, dtype=np.float32).reshape(x.shape[0], -1)[:, :1]
        v = (x * s0 + s1).astype(np.float32)
        o1 = (v * v).astype(np.float32)
        v2 = (o1 + bb).astype(np.float32)
        return (v2 * v2).astype(np.float32)

    body = _spill_c3_to_src1(sq(sq(Src0 * C0 + C1) + C3))
    spec = Spec(body=body, reference=_ref)
    row = max(dvo._SUB_OPCODE_FOR_NAME.values()) + 1
    uops = lower(spec, ver="v3")
    sha = DveOpSpec(name=name, opcode=row, uops=uops, rd1_en=_has_src1(spec)).sha("v3")
    op = dvo.DveOp(name=name, spec=spec, subdim=False, uops_sha={"v3": sha})
    dvo.OPS.append(op)
    dvo._SUB_OPCODE_FOR_NAME[name] = row
    dvo.CUSTOM_DVE_SPECS[name] = spec
    return op


def register_triple_op():
    """out = sq(sq(sq(Src0*C0 + C1) + L1) + L2): THREE fused layers. L1/L2 are
    two per-partition scalars streamed from Src1 (in1 = [P,2]); the single
    latch-init state lower() produces (which would latch the same Src1 element
    into both swap flops) is split into two one-cycle states so stage-3 and
    stage-5 latch consecutive Src1 elements."""
    import copy as _copy

    _import_concourse()
    from concourse import dve_ops as dvo
    from concourse.dve_spec import C0, C1, Latch, Spec, Src0, Src1, lower, sq
    from concourse.dve_uop import DveOpSpec

    name = "SQ_TRIPLE_ANT"
    for op in dvo.OPS:
        if op.name == name:
            return op

    def _ref(in0, in1, s0, s1, imm2):
        x = in0.astype(np.float32)
        bb = np.asarray(in1, dtype=np.float32).reshape(x.shape[0], -1)
        v = (x * s0 + s1).astype(np.float32)
        o = (v * v).astype(np.float32)
        v = (o + bb[:, 0:1]).astype(np.float32)
        o = (v * v).astype(np.float32)
        v = (o + bb[:, 1:2]).astype(np.float32)
        return (v * v).astype(np.float32)

    body = sq(sq(sq(Src0 * C0 + C1) + Latch(Src1)) + Latch(Src1))
    spec = Spec(body=body, reference=_ref)
    uops = lower(spec, ver="v3")
    assert len(uops) == 2, f"expected [latch-init, steady], got {len(uops)}"
    li, steady = uops
    swap_stages = [
        i for i, b in enumerate(li.datapath_config) if b.swap_enable
    ]
    assert len(swap_stages) == 2, swap_stages
    li_a = _copy.deepcopy(li)
    li_a.datapath_config[swap_stages[1]].swap_enable = 0
    li_b = _copy.deepcopy(li)
    li_b.datapath_config[swap_stages[0]].swap_enable = 0
    li_b.next_uop = (2, 0, 0)
    uops3 = [li_a, li_b, steady]

    row = max(dvo._SUB_OPCODE_FOR_NAME.values()) + 1
    dspec = DveOpSpec(name=name, opcode=row, uops=uops3, rd1_en=True)
    op = dvo.DveOp(name=name, spec=spec, subdim=False, uops_sha={"v3": dspec.sha("v3")})
    dvo.OPS.append(op)
    dvo._SUB_OPCODE_FOR_NAME[name] = row
    dvo.CUSTOM_DVE_SPECS[name] = spec
    dvo._COMPILE_CACHE[(name, "v3")] = dspec
    return op


def register_scan_op(name, alu_name, init_name):
    """f32 min/max scan with accum, with an authored 2X_2P perf variant
    (2 elem/cycle via both SBUF read ports; mirrors the stock tensor_scalar
    2X_2P control conventions). out = in (passthrough), accum_out = min/max."""
    import copy as _copy

    _import_concourse()
    from concourse import dve_ops as dvo
    from concourse.dve_spec import Leaf, Spec, Src0, lower
    from concourse.dve_uop import AluInp, AluOp as UAlu, DveOpSpec, InpSel, OutPath, OutSel

    for op in dvo.OPS:
        if op.name == name:
            return op
    alu = getattr(UAlu, alu_name)
    init_sel = getattr(InpSel, init_name)
    spec = Spec(body=Src0, accum=alu, accum_init=Leaf(init_sel))
    uops1x = lower(spec, ver="v3")
    assert len(uops1x) == 2
    seed2p = _copy.deepcopy(uops1x[0])
    st = _copy.deepcopy(uops1x[1])
    st.enable_input(InpSel.SRC_1, 3)  # second stream on lane 2
    st.require_inp1 = 1
    for b in st.datapath_config:
        b.pass_through_delay(2)
    st.datapath_config[0].enable_alu(alu, AluInp.PREV_DELAY_0, AluInp.PREV_DELAY_2)
    st.enable_output(OutSel.DELAY_2, OutPath.WR1_LO)
    uops2p = [seed2p, st]
    row = max(dvo._SUB_OPCODE_FOR_NAME.values()) + 1
    dspec = DveOpSpec(
        name=name,
        opcode=row,
        uops=uops1x,
        uops_2x=uops2p,
        uops_2x_2p=uops2p,
        uops_4x=None,
        perf_max=2,
        rd1_en=False,
    )
    op = dvo.DveOp(name=name, spec=spec, subdim=False, uops_sha={"v3": dspec.sha("v3")})
    dvo.OPS.append(op)
    dvo._SUB_OPCODE_FOR_NAME[name] = row
    dvo.CUSTOM_DVE_SPECS[name] = spec
    dvo._COMPILE_CACHE[(name, "v3")] = dspec
    return op


def build_nc(enable_asserts=False):
    _import_concourse()
    import concourse.bacc as bacc
    import concourse.tile as tile
    from concourse import bass_isa, mybir

    pair_op = register_pair_op()
    triple_op = register_triple_op()
    scan_min = register_scan_op("SCAN_MIN_2P_ANT", "MIN", "MAX_POS")
    scan_max = register_scan_op("SCAN_MAX_2P_ANT", "MAX", "MAX_NEG")

    f32 = mybir.dt.float32
    Alu = mybir.AluOpType
    Act = mybir.ActivationFunctionType
    AX = mybir.AxisListType

    nc = bacc.Bacc(
        "TRN2",
        target_bir_lowering=False,
        debug=False,
        enable_asserts=enable_asserts,
        num_devices=N_CORES,
    )

    xs = nc.dram_tensor("xs", [CH_PER_CORE, 128, F_FULL], f32, kind="ExternalInput").ap()
    coef = nc.dram_tensor("coef", [128, NCOEF], f32, kind="ExternalInput").ap()
    ys = nc.dram_tensor("ys", [CH_PER_CORE, 128, F_FULL], f32, kind="ExternalOutput").ap()

    with tile.TileContext(nc) as tc:
        with (
            tc.tile_pool(name="data", bufs=3) as dpool,
            tc.tile_pool(name="cst", bufs=1) as cpool,
            tc.tile_pool(name="st", bufs=2) as st,
            tc.tile_pool(name="pt", bufs=4) as pt,
        ):
            coeft = cpool.tile([128, NCOEF], f32, tag="coeft", name="coeft")
            nc.sync.dma_start(out=coeft[:], in_=coef)

            def cv(nm, l):
                base = _COEF_NAMES.index(nm) * _NL4 + l * CH_PER_CORE
                return coeft[:, base : base + CH_PER_CORE]

            cf0v = coeft[:, len(_COEF_NAMES) * _NL4 :]

            # 7 groups: 6 triples (layers 3g..3g+2) + 1 final pair (18,19)
            N_GROUPS = 7
            alphaT = cpool.tile([128, N_GROUPS * 4], f32, tag="alphaT", name="alphaT")
            betaT = cpool.tile([128, N_GROUPS * 4], f32, tag="betaT", name="betaT")
            bb2T = cpool.tile([128, 6 * 8], f32, tag="bb2T", name="bb2T")
            betabT = cpool.tile([128, 4], f32, tag="betabT", name="betabT")
            cf1T = cpool.tile([128, 4], f32, tag="cf1T", name="cf1T")
            mn0t = cpool.tile([128, 4], f32, tag="mn0t", name="mn0t")
            mx0t = cpool.tile([128, 4], f32, tag="mx0t", name="mx0t")

            def s4(tag):
                return st.tile([128, 4], f32, tag=tag, name=tag)

            # ---------- Phase 1: DMA in + layer-0 min/max scans ----------
            def scan_chunk(src_chunk, c, k, pmn, pmx):
                emit_scan(nc, scan_min, src_chunk, src_chunk, pmn[:, k : k + 1])
                emit_scan(nc, scan_max, src_chunk, src_chunk, pmx[:, k : k + 1])

            def combine(c, pmn, pmx):
                rmn = pt.tile([128, 1], f32, tag="rmn", name="rmn")
                rmx = pt.tile([128, 1], f32, tag="rmx", name="rmx")
                nc.vector.tensor_reduce(rmn[:], pmn[:], axis=AX.X, op=Alu.min)
                nc.vector.tensor_reduce(rmx[:], pmx[:], axis=AX.X, op=Alu.max)
                nc.vector.tensor_scalar_mul(rmn[:], rmn[:], -1.0)
                nmn = pt.tile([128, 1], f32, tag="nmn", name="nmn")
                nc.gpsimd.partition_all_reduce(nmn[:], rmn[:], 128, bass_isa.ReduceOp.max)
                nc.vector.tensor_scalar_mul(mn0t[:, c : c + 1], nmn[:], -1.0)
                nc.gpsimd.partition_all_reduce(
                    mx0t[:, c : c + 1], rmx[:], 128, bass_isa.ReduceOp.max
                )

            # ch3 stream-scanned via two chunk-scratch acquisitions (slots 0,1)
            scr = [
                dpool.tile([128, CW], f32, tag="W", name=f"scr{i}") for i in range(2)
            ]
            pmn3 = pt.tile([128, NCHUNK], f32, tag="pmn", name="pmn3")
            pmx3 = pt.tile([128, NCHUNK], f32, tag="pmx", name="pmx3")
            for k in range(NCHUNK):
                s = scr[k % 2]
                nc.sync.dma_start(out=s[:], in_=xs[3][:, k * CW : (k + 1) * CW])
                scan_chunk(s[:], 3, k, pmn3, pmx3)

            W = {}
            for c in range(3):
                W[c] = dpool.tile([128, F_FULL], f32, tag="W", name=f"W{c}")
                pmn = pt.tile([128, NCHUNK], f32, tag="pmn", name=f"pmn{c}")
                pmx = pt.tile([128, NCHUNK], f32, tag="pmx", name=f"pmx{c}")
                for k in range(NCHUNK):
                    ck = W[c][:, k * CW : (k + 1) * CW]
                    nc.sync.dma_start(out=ck, in_=xs[c][:, k * CW : (k + 1) * CW])
                    scan_chunk(ck, c, k, pmn, pmx)
                combine(c, pmn, pmx)
            combine(3, pmn3, pmx3)

            # ---------- Phase 2+3 interleaved: chain (1 pair lookahead) + units
            # boot
            D0 = s4("D0")
            nc.vector.tensor_sub(D0[:], mx0t[:], mn0t[:])
            Dse0 = s4("Dse0")
            nc.vector.tensor_scalar_add(Dse0[:], D0[:], EPS)
            sp0 = s4("sp0")
            nc.vector.reciprocal(sp0[:], Dse0[:])
            A = s4("A")
            nc.vector.tensor_scalar(A[:], sp0[:], -EPS, 1.0, Alu.mult, Alu.add)
            rgamma = sp0
            delta = mn0t

            state = {"A": A, "rgamma": rgamma, "delta": delta, "gmid": None, "dmid": None}

            def chain_layer_stats(l):
                t1 = s4("t1")
                nc.vector.tensor_add(t1[:], state["A"][:], cv("d2", l))
                eA = s4("eA")
                nc.vector.tensor_mul(eA[:], t1[:], t1[:])
                i_ = s4("i_")
                nc.vector.scalar_tensor_tensor(
                    i_[:], t1[:], 0.0, cv("nf", l), Alu.is_gt, Alu.mult
                )
                j = s4("j")
                nc.vector.tensor_scalar(j[:], i_[:], -1.0, 1.0, Alu.mult, Alu.add)
                mne = s4("mne")
                nc.vector.tensor_tensor(mne[:], eA[:], cv("e0", l), Alu.min)
                mn = s4("mn")
                nc.vector.tensor_mul(mn[:], mne[:], j[:])
                mx = s4("mx")
                nc.vector.tensor_tensor(mx[:], eA[:], cv("e0", l), Alu.max)
                spr = s4("spr")
                nc.vector.tensor_sub(spr[:], mx[:], mn[:])
                tg = s4("tg")
                nc.vector.tensor_mul(tg[:], spr[:], cv("g", l))
                E = s4("E")
                nc.vector.tensor_sub(E[:], mx[:], tg[:])
                tD = s4("tD")
                nc.vector.tensor_mul(tD[:], spr[:], cv("absa2", l))
                Dse = s4("Dse")
                nc.vector.tensor_scalar_add(Dse[:], tD[:], EPS)
                sp = s4("sp")
                nc.vector.reciprocal(sp[:], Dse[:])
                Anew = s4("Anew")
                nc.vector.tensor_scalar(Anew[:], sp[:], -EPS, 1.0, Alu.mult, Alu.add)
                state["A"] = Anew
                return E, Dse, sp

            def chain_A_layer(lA, av, bv):
                """scaled layer: writes alpha/beta; returns (gmid, dmid)."""
                E, Dse, sp = chain_layer_stats(lA)
                aspa = s4("aspa")
                nc.vector.tensor_mul(aspa[:], cv("absa2", lA), sp[:])
                w = s4("w")
                nc.scalar.activation(w[:], aspa[:], Act.Sqrt)
                w2 = s4("w2")
                nc.vector.tensor_mul(w2[:], w[:], w[:])
                raspa = s4("raspa")
                nc.vector.tensor_mul(raspa[:], Dse[:], cv("r_absa2", lA))
                gmu = s4("gmu")
                nc.vector.tensor_mul(gmu[:], w2[:], raspa[:])
                gmid = s4("gmid")
                nc.vector.tensor_mul(gmid[:], gmu[:], cv("sgn", lA))
                nc.vector.tensor_mul(av, w[:], state["rgamma"][:])
                tad = s4("tad")
                nc.vector.tensor_mul(tad[:], av, state["delta"][:])
                twd = s4("twd")
                nc.vector.tensor_mul(twd[:], w[:], cv("d2", lA))
                nc.vector.tensor_sub(bv, twd[:], tad[:])
                dmid = s4("dmid")
                nc.vector.tensor_mul(dmid[:], w2[:], E[:])
                return gmid, dmid

            def chain_unit_layer(l, gam_in, del_in, bbv):
                """unit layer: Z' = (Z + bb)^2 given input affine (gam, del).
                Writes bb; returns (gam_out, del_out)."""
                E2, Dse2, _ = chain_layer_stats(l)
                tbd = s4("tbd")
                nc.vector.tensor_mul(tbd[:], gam_in[:], cv("d2", l))
                nc.vector.tensor_sub(bbv, tbd[:], del_in[:])
                gm2 = s4("gm2")
                nc.vector.tensor_mul(gm2[:], gam_in[:], gam_in[:])
                tg2 = s4("tg2")
                nc.vector.tensor_mul(tg2[:], gm2[:], Dse2[:])
                gam = s4("gam")
                nc.vector.tensor_mul(gam[:], tg2[:], cv("r_a2", l))
                dele = s4("dele")
                nc.vector.tensor_mul(dele[:], gm2[:], E2[:])
                return gam, dele, gm2

            def chain_group(g):
                av = alphaT[:, g * 4 : g * 4 + 4]
                bv = betaT[:, g * 4 : g * 4 + 4]
                if g < 6:
                    lA = 3 * g
                    gmid, dmid = chain_A_layer(lA, av, bv)
                    bbB = bb2T[:, g * 8 + 0 : g * 8 + 8 : 2]
                    bbC = bb2T[:, g * 8 + 1 : g * 8 + 8 : 2]
                    gamB, delB, _ = chain_unit_layer(lA + 1, gmid, dmid, bbB)
                    gamC, delC, _ = chain_unit_layer(lA + 2, gamB, delB, bbC)
                    rg = s4("rg")
                    nc.vector.reciprocal(rg[:], gamC[:])
                    state["rgamma"] = rg
                    state["delta"] = delC
                else:
                    lA = 18
                    gmid, dmid = chain_A_layer(lA, av, bv)
                    E2, Dse2, _ = chain_layer_stats(19)
                    tbd = s4("tbd")
                    nc.vector.tensor_mul(tbd[:], gmid[:], cv("d2", 19))
                    nc.vector.tensor_sub(betabT[:], tbd[:], dmid[:])
                    gm2 = s4("gm2")
                    nc.vector.tensor_mul(gm2[:], gmid[:], gmid[:])
                    rgm2 = s4("rgm2")
                    nc.vector.reciprocal(rgm2[:], gm2[:])
                    a2c = s4("a2c")
                    nc.vector.tensor_mul(a2c[:], cv("absa2", 19), cv("sgn", 19))
                    nc.vector.tensor_mul(cf1T[:], a2c[:], rgm2[:])

            def unit(c, k, g):
                ck = W[c][:, k * CW : (k + 1) * CW]
                a_ap = alphaT[:, g * 4 + c : g * 4 + c + 1]
                b_ap = betaT[:, g * 4 + c : g * 4 + c + 1]
                if g < 6:
                    bb_pair = bb2T[:, g * 8 + 2 * c : g * 8 + 2 * c + 2]
                    if unit_engine_is_act(c, k, g):
                        nc.scalar.activation(ck, ck, Act.Square, bias=b_ap, scale=a_ap)
                        nc.scalar.activation(
                            ck, ck, Act.Square,
                            bias=bb2T[:, g * 8 + 2 * c : g * 8 + 2 * c + 1], scale=1.0,
                        )
                        nc.scalar.activation(
                            ck, ck, Act.Square,
                            bias=bb2T[:, g * 8 + 2 * c + 1 : g * 8 + 2 * c + 2], scale=1.0,
                        )
                    else:
                        nc.vector._custom_dve(
                            triple_op, out=ck, in0=ck, in1=bb_pair, s0=a_ap, s1=b_ap
                        )
                else:
                    bb_ap = betabT[:, c : c + 1]
                    if unit_engine_is_act(c, k, g):
                        nc.scalar.activation(ck, ck, Act.Square, bias=b_ap, scale=a_ap)
                        nc.scalar.activation(ck, ck, Act.Square, bias=bb_ap, scale=1.0)
                    else:
                        nc.vector._custom_dve(
                            pair_op, out=ck, in0=ck, in1=bb_ap, s0=a_ap, s1=b_ap
                        )

            def finish_chunk(c, k):
                ck = W[c][:, k * CW : (k + 1) * CW]
                cf1_ap = cf1T[:, c : c + 1]
                cf0_ap = cf0v[:, c : c + 1]
                if affine_engine_is_act(c, k):
                    nc.scalar.activation(ck, ck, Act.Identity, bias=cf0_ap, scale=cf1_ap)
                else:
                    nc.vector.tensor_scalar(ck, ck, cf1_ap, cf0_ap, Alu.mult, Alu.add)
                nc.sync.dma_start(out=ys[c][:, k * CW : (k + 1) * CW], in_=ck)

            # chain runs 1 group ahead of the units of channels 0-2
            chain_group(0)
            for g in range(N_GROUPS):
                if g + 1 < N_GROUPS:
                    chain_group(g + 1)
                for c in range(3):
                    for k in range(NCHUNK):
                        unit(c, k, g)
            for c in range(3):
                for k in range(NCHUNK):
                    finish_chunk(c, k)

            # ---------- Phase 3b: channel 3 (buffer freed by ch0) ----------
            W[3] = dpool.tile([128, F_FULL], f32, tag="W", name="W3")
            for k in range(NCHUNK):
                nc.sync.dma_start(
                    out=W[3][:, k * CW : (k + 1) * CW],
                    in_=xs[3][:, k * CW : (k + 1) * CW],
                )
            for k in range(NCHUNK):
                for g in range(N_GROUPS):
                    unit(3, k, g)
                finish_chunk(3, k)

    nc.compile()
    return nc


_NC_CACHE = {}


def _get_nc():
    if "full" not in _NC_CACHE:
        _NC_CACHE["full"] = build_nc()
    return _NC_CACHE["full"]


def host_coefs(w0, w1, w2):
    """Per-core coef arrays [128, NCOEF] (f32, broadcast over partitions)."""
    f = np.float32
    a2 = np.asarray(w2, dtype=f)
    a1 = np.asarray(w1, dtype=f)
    a0 = np.asarray(w0, dtype=f)
    sgn = np.where(a2 >= 0, f(1), f(-1)).astype(f)
    a2cl = (sgn * np.maximum(np.abs(a2), f(CLAMP))).astype(f)
    d2 = (a1 / a2cl / 2).astype(f)
    e0 = (d2 * d2).astype(f)
    nf = (d2 < 0).astype(f)
    g = (a2cl >= 0).astype(f)
    absa2 = np.abs(a2cl).astype(f)
    r_a2 = (f(1) / a2cl).astype(f)
    r_absa2 = (f(1) / absa2).astype(f)
    arrays = {
        "d2": d2, "e0": e0, "nf": nf, "g": g,
        "absa2": absa2, "r_a2": r_a2, "sgn": sgn, "r_absa2": r_absa2,
    }
    cf0 = (a0[N_LAYERS - 1] - a2cl[N_LAYERS - 1] * e0[N_LAYERS - 1]).astype(f)

    out = []
    for core in range(N_CORES):
        cols = slice(CH_PER_CORE * core, CH_PER_CORE * (core + 1))
        row = np.empty(NCOEF, dtype=f)
        for idx, nm in enumerate(_COEF_NAMES):
            arr = arrays[nm][:, cols]  # [NL, 4]
            row[idx * _NL4 : (idx + 1) * _NL4] = arr.reshape(-1)  # l*4+c
        row[len(_COEF_NAMES) * _NL4 :] = cf0[cols]
        out.append(np.ascontiguousarray(np.broadcast_to(row[None, :], (128, NCOEF))))
    return out


def shard_inputs(x, w0, w1, w2):
    x = np.ascontiguousarray(x, dtype=np.float32)
    coefs = host_coefs(w0, w1, w2)
    in_maps = []
    for k in range(N_CORES):
        cols = slice(CH_PER_CORE * k, CH_PER_CORE * (k + 1))
        xk = np.ascontiguousarray(x[:, cols].transpose(1, 0, 2, 3)).reshape(
            CH_PER_CORE, 128, F_FULL
        )
        in_maps.append({"xs": xk, "coef": coefs[k]})
    return in_maps


def unshard_output(results):
    out = np.empty((B, C, H, Wd), dtype=np.float32)
    for k in range(N_CORES):
        ysk = np.asarray(results[k]["ys"], dtype=np.float32).reshape(
            CH_PER_CORE, B, H, Wd
        )
        out[:, CH_PER_CORE * k : CH_PER_CORE * (k + 1)] = ysk.transpose(1, 0, 2, 3)
    return out


def run_sharded(in_maps, trace=False, trace_kwargs=None):
    _import_concourse()
    from concourse.bass_utils import run_bass_kernel_spmd

    nc = _get_nc()
    return run_bass_kernel_spmd(
        nc,
        in_maps,
        core_ids=list(range(N_CORES)),
        trace=trace,
        **(trace_kwargs or {}),
    )


def kernel(x, w0, w1, w2):
    in_maps = shard_inputs(x, w0, w1, w2)
    res = run_sharded(in_maps)
    return unshard_output(res.results)


# revision 18
# speedup vs baseline: 1.5343x; 1.1176x over previous
"""Trainium2 Bass kernel for nn_ActSeries: 20 layers of per-channel range-norm +
quadratic polynomial, x [32,32,256,256] f32.

Strategy (v2 — analytic range propagation, dual-engine streaming)
-----------------------------------------------------------------
Shard the 32 channels across 8 cores (4 channels/core); per-channel stats make
every reduction core-local (no collectives).

Math: each layer is h' = a2*xh^2 + a1*xh + a0 with xh = (h-mn)/(mx-mn+eps).
Complete the square: h' = a2*(xh + d2)^2 + const, d2 = a1/(2*a2). The range-norm
is invariant to tracked affine maps, so we store Z = gamma*xh + delta and fold
each layer into Z' = (alpha*Z + beta)^2 (one multiply-add-square per element).
Key observation: the data min/max of the NEXT layer is analytic given this
layer's range [0, A]: max over the interval is attained at an endpoint (both
endpoints ARE data points), and the interior-vertex min is ~0 to within the
data spacing squared (~1e-12), far below the 2e-2 tolerance. So after a single
min/max scan of the raw input (layer 0), all 20 layers' scale/offset constants
follow from a tiny per-channel scalar recurrence — no more data scans, no
inter-layer dependencies beyond the elementwise stream.

Per-pair affine normalization: the A-layer (even) picks its output scale
w = sqrt(|a2*s'|) so gamma_mid = +-1; the B-layer (odd) then needs no scale:
Z'' = (Z' + betab)^2. Two layers fuse into ONE 5-stage custom DVE op
  out = sq(sq(Src0*C0 + C1) + C3)   (C0=alpha, C1=beta, C3=betab via Src1 latch)
at 1 elem/cycle, i.e. 2 layer-elements/cycle. The Scalar engine computes the
same layers via ACTIVATE Square ((scale*x+bias)^2), so DVE and ACT split the
chunks ~5:3 and run concurrently. Final y = cf1*Z + cf0 in one affine pass.
Everything runs in place (verified on HW); 3 channel buffers rotate in SBUF.

Validated end-to-end in numpy against the reference: rel err ~2e-4.
"""

import os
import sys

import numpy as np

B, C, H, Wd = 32, 32, 256, 256
N_LAYERS = 20
N_PAIRS = N_LAYERS // 2
EPS = 1e-5
N_CORES = 8
CH_PER_CORE = C // N_CORES  # 4
F_FULL = B * H * Wd // 128  # 16384 free-dim elements per partition
CW = 4096
NCHUNK = F_FULL // CW  # 4
CLAMP = 1e-4  # |a2| clamp; error bounded by CLAMP*A^2 << tol

# Engine assignment: channels 0-2 chunk-static (ACT takes chunks {1,3});
# channel 3 (the deferred-buffer channel) is pair-level mixed so both engines
# share its tail. Ratio tuned for DVE pair-op 4.54us vs ACT 2-activate 7.0us.
def unit_engine_is_act(c, k, p):
    return (c, k) in {(0, 2), (1, 1), (1, 3), (2, 2), (3, 1)}


def affine_engine_is_act(c, k):
    return unit_engine_is_act(c, k, 0)


def emit_scan(nc, op, out, in0, accum_out):
    """Emit a scan custom-DVE op with perf_max=2 so the engine may select the
    authored 2X_2P program (perf_max must be set at construction — the 64B
    instruction image is encoded eagerly)."""
    _import_concourse()
    from concourse import bass_isa, mybir
    from concourse.dve_ops import get_dve_sub_opcode

    vec = nc.vector
    if op.name not in vec.bass.m.ant_custom_dve_ops:
        vec.bass.m.ant_custom_dve_ops = sorted(
            {*vec.bass.m.ant_custom_dve_ops, op.name}
        )
    op.compile("v3")
    shape = bass_isa.CustomDveShape.TTSS
    isa_opcode = vec.bass.isa.Opcode[
        f"NEURON_ISA_TPB_OPCODE_CUSTOM_DVE_ANT_{shape.slot()}"
    ].value
    imm0 = mybir.ImmediateValue(dtype=mybir.dt.float32, value=0.0)
    ins = [vec.lower_ap(in0, for_isa=True, opt=True), imm0, imm0]
    outs = [
        vec.lower_ap(out, for_isa=True, opt=True),
        vec.lower_ap(accum_out, for_isa=True),
    ]
    return vec.add_instruction(
        bass_isa.InstCustomDveAnt(
            name=vec.bass.get_next_instruction_name(),
            op_name=op.name,
            rd1_en=False,
            subdim=0,
            imm2=0.0,
            shape=shape,
            row=get_dve_sub_opcode(op.name),
            isa_opcode=isa_opcode,
            ins=ins,
            outs=outs,
            perf_max=2,
        )
    )

# coef column layout: 8 per-layer arrays of [N_LAYERS*4] (l*4+c), then cf0 [4]
_NL4 = N_LAYERS * CH_PER_CORE  # 80
_COEF_NAMES = ("d2", "e0", "nf", "g", "absa2", "r_a2", "sgn", "r_absa2")
NCOEF = len(_COEF_NAMES) * _NL4 + CH_PER_CORE  # 644


def _import_concourse():
    try:
        import concourse  # noqa: F401
    except ImportError:
        for p in ("/opt/trn_rl_repo", os.path.expanduser("~/.axon_site/_ro/trn_rl_repo")):
            if os.path.isdir(p) and p not in sys.path:
                sys.path.insert(0, p)
        import concourse  # noqa: F401


def register_pair_op():
    """out = sq(sq(Src0*C0 + C1) + C3): two fused layers, C3 spilled to Src1."""
    _import_concourse()
    from concourse import dve_ops as dvo
    from concourse.dve_spec import (
        C0,
        C1,
        C3,
        Spec,
        Src0,
        _has_src1,
        _spill_c3_to_src1,
        lower,
        sq,
    )
    from concourse.dve_uop import DveOpSpec

    name = "SQ_PAIR_ANT"
    for op in dvo.OPS:
        if op.name == name:
            return op

    def _ref(in0, in1, s0, s1, imm2):
        x = in0.astype(np.float32)
        bb = np.asarray(in1, dtype=np.float32).reshape(x.shape[0], -1)[:, :1]
        v = (x * s0 + s1).astype(np.float32)
        o1 = (v * v).astype(np.float32)
        v2 = (o1 + bb).astype(np.float32)
        return (v2 * v2).astype(np.float32)

    body = _spill_c3_to_src1(sq(sq(Src0 * C0 + C1) + C3))
    spec = Spec(body=body, reference=_ref)
    row = max(dvo._SUB_OPCODE_FOR_NAME.values()) + 1
    uops = lower(spec, ver="v3")
    sha = DveOpSpec(name=name, opcode=row, uops=uops, rd1_en=_has_src1(spec)).sha("v3")
    op = dvo.DveOp(name=name, spec=spec, subdim=False, uops_sha={"v3": sha})
    dvo.OPS.append(op)
    dvo._SUB_OPCODE_FOR_NAME[name] = row
    dvo.CUSTOM_DVE_SPECS[name] = spec
    return op


def register_triple_op():
    """out = sq(sq(sq(Src0*C0 + C1) + L1) + L2): THREE fused layers. L1/L2 are
    two per-partition scalars streamed from Src1 (in1 = [P,2]); the single
    latch-init state lower() produces (which would latch the same Src1 element
    into both swap flops) is split into two one-cycle states so stage-3 and
    stage-5 latch consecutive Src1 elements."""
    import copy as _copy

    _import_concourse()
    from concourse import dve_ops as dvo
    from concourse.dve_spec import C0, C1, Latch, Spec, Src0, Src1, lower, sq
    from concourse.dve_uop import DveOpSpec

    name = "SQ_TRIPLE_ANT"
    for op in dvo.OPS:
        if op.name == name:
            return op

    def _ref(in0, in1, s0, s1, imm2):
        x = in0.astype(np.float32)
        bb = np.asarray(in1, dtype=np.float32).reshape(x.shape[0], -1)
        v = (x * s0 + s1).astype(np.float32)
        o = (v * v).astype(np.float32)
        v = (o + bb[:, 0:1]).astype(np.float32)
        o = (v * v).astype(np.float32)
        v = (o + bb[:, 1:2]).astype(np.float32)
        return (v * v).astype(np.float32)

    body = sq(sq(sq(Src0 * C0 + C1) + Latch(Src1)) + Latch(Src1))
    spec = Spec(body=body, reference=_ref)
    uops = lower(spec, ver="v3")
    assert len(uops) == 2, f"expected [latch-init, steady], got {len(uops)}"
    li, steady = uops
    swap_stages = [
        i for i, b in enumerate(li.datapath_config) if b.swap_enable
    ]
    assert len(swap_stages) == 2, swap_stages
    li_a = _copy.deepcopy(li)
    li_a.datapath_config[swap_stages[1]].swap_enable = 0
    li_b = _copy.deepcopy(li)
    li_b.datapath_config[swap_stages[0]].swap_enable = 0
    li_b.next_uop = (2, 0, 0)
    uops3 = [li_a, li_b, steady]

    row = max(dvo._SUB_OPCODE_FOR_NAME.values()) + 1
    dspec = DveOpSpec(name=name, opcode=row, uops=uops3, rd1_en=True)
    op = dvo.DveOp(name=name, spec=spec, subdim=False, uops_sha={"v3": dspec.sha("v3")})
    dvo.OPS.append(op)
    dvo._SUB_OPCODE_FOR_NAME[name] = row
    dvo.CUSTOM_DVE_SPECS[name] = spec
    dvo._COMPILE_CACHE[(name, "v3")] = dspec
    return op


def register_scan_op(name, alu_name, init_name):
    """f32 min/max scan with accum, with an authored 2X_2P perf variant
    (2 elem/cycle via both SBUF read ports; mirrors the stock tensor_scalar
    2X_2P control conventions). out = in (passthrough), accum_out = min/max."""
    import copy as _copy

    _import_concourse()
    from concourse import dve_ops as dvo
    from concourse.dve_spec import Leaf, Spec, Src0, lower
    from concourse.dve_uop import AluInp, AluOp as UAlu, DveOpSpec, InpSel, OutPath, OutSel

    for op in dvo.OPS:
        if op.name == name:
            return op
    alu = getattr(UAlu, alu_name)
    init_sel = getattr(InpSel, init_name)
    spec = Spec(body=Src0, accum=alu, accum_init=Leaf(init_sel))
    uops1x = lower(spec, ver="v3")
    assert len(uops1x) == 2
    seed2p = _copy.deepcopy(uops1x[0])
    st = _copy.deepcopy(uops1x[1])
    st.enable_input(InpSel.SRC_1, 3)  # second stream on lane 2
    st.require_inp1 = 1
    for b in st.datapath_config:
        b.pass_through_delay(2)
    st.datapath_config[0].enable_alu(alu, AluInp.PREV_DELAY_0, AluInp.PREV_DELAY_2)
    st.enable_output(OutSel.DELAY_2, OutPath.WR1_LO)
    uops2p = [seed2p, st]
    row = max(dvo._SUB_OPCODE_FOR_NAME.values()) + 1
    dspec = DveOpSpec(
        name=name,
        opcode=row,
        uops=uops1x,
        uops_2x=uops2p,
        uops_2x_2p=uops2p,
        uops_4x=None,
        perf_max=2,
        rd1_en=False,
    )
    op = dvo.DveOp(name=name, spec=spec, subdim=False, uops_sha={"v3": dspec.sha("v3")})
    dvo.OPS.append(op)
    dvo._SUB_OPCODE_FOR_NAME[name] = row
    dvo.CUSTOM_DVE_SPECS[name] = spec
    dvo._COMPILE_CACHE[(name, "v3")] = dspec
    return op


def build_nc(enable_asserts=False):
    _import_concourse()
    import concourse.bacc as bacc
    import concourse.tile as tile
    from concourse import bass_isa, mybir

    pair_op = register_pair_op()
    triple_op = register_triple_op()
    scan_min = register_scan_op("SCAN_MIN_2P_ANT", "MIN", "MAX_POS")
    scan_max = register_scan_op("SCAN_MAX_2P_ANT", "MAX", "MAX_NEG")

    f32 = mybir.dt.float32
    Alu = mybir.AluOpType
    Act = mybir.ActivationFunctionType
    AX = mybir.AxisListType

    nc = bacc.Bacc(
        "TRN2",
        target_bir_lowering=False,
        debug=False,
        enable_asserts=enable_asserts,
        num_devices=N_CORES,
    )

    xs = nc.dram_tensor("xs", [CH_PER_CORE, 128, F_FULL], f32, kind="ExternalInput").ap()
    coef = nc.dram_tensor("coef", [128, NCOEF], f32, kind="ExternalInput").ap()
    ys = nc.dram_tensor("ys", [CH_PER_CORE, 128, F_FULL], f32, kind="ExternalOutput").ap()

    with tile.TileContext(nc) as tc:
        with (
            tc.tile_pool(name="data", bufs=3) as dpool,
            tc.tile_pool(name="cst", bufs=1) as cpool,
            tc.tile_pool(name="st", bufs=2) as st,
            tc.tile_pool(name="pt", bufs=4) as pt,
        ):
            coeft = cpool.tile([128, NCOEF], f32, tag="coeft", name="coeft")
            nc.sync.dma_start(out=coeft[:], in_=coef)

            def cv(nm, l):
                base = _COEF_NAMES.index(nm) * _NL4 + l * CH_PER_CORE
                return coeft[:, base : base + CH_PER_CORE]

            cf0v = coeft[:, len(_COEF_NAMES) * _NL4 :]

            # 7 groups: 6 triples (layers 3g..3g+2) + 1 final pair (18,19)
            N_GROUPS = 7
            alphaT = cpool.tile([128, N_GROUPS * 4], f32, tag="alphaT", name="alphaT")
            betaT = cpool.tile([128, N_GROUPS * 4], f32, tag="betaT", name="betaT")
            bb2T = cpool.tile([128, 6 * 8], f32, tag="bb2T", name="bb2T")
            betabT = cpool.tile([128, 4], f32, tag="betabT", name="betabT")
            cf1T = cpool.tile([128, 4], f32, tag="cf1T", name="cf1T")
            mn0t = cpool.tile([128, 4], f32, tag="mn0t", name="mn0t")
            mx0t = cpool.tile([128, 4], f32, tag="mx0t", name="mx0t")

            def s4(tag):
                return st.tile([128, 4], f32, tag=tag, name=tag)

            # ---------- Phase 1: DMA in + layer-0 min/max scans ----------
            def scan_chunk(src_chunk, c, k, pmn, pmx):
                emit_scan(nc, scan_min, src_chunk, src_chunk, pmn[:, k : k + 1])
                emit_scan(nc, scan_max, src_chunk, src_chunk, pmx[:, k : k + 1])

            def combine(c, pmn, pmx):
                rmn = pt.tile([128, 1], f32, tag="rmn", name="rmn")
                rmx = pt.tile([128, 1], f32, tag="rmx", name="rmx")
                nc.vector.tensor_reduce(rmn[:], pmn[:], axis=AX.X, op=Alu.min)
                nc.vector.tensor_reduce(rmx[:], pmx[:], axis=AX.X, op=Alu.max)
                nc.vector.tensor_scalar_mul(rmn[:], rmn[:], -1.0)
                nmn = pt.tile([128, 1], f32, tag="nmn", name="nmn")
                nc.gpsimd.partition_all_reduce(nmn[:], rmn[:], 128, bass_isa.ReduceOp.max)
                nc.vector.tensor_scalar_mul(mn0t[:, c : c + 1], nmn[:], -1.0)
                nc.gpsimd.partition_all_reduce(
                    mx0t[:, c : c + 1], rmx[:], 128, bass_isa.ReduceOp.max
                )

            # ch3 stream-scanned via two chunk-scratch acquisitions (slots 0,1)
            scr = [
                dpool.tile([128, CW], f32, tag="W", name=f"scr{i}") for i in range(2)
            ]
            pmn3 = pt.tile([128, NCHUNK], f32, tag="pmn", name="pmn3")
            pmx3 = pt.tile([128, NCHUNK], f32, tag="pmx", name="pmx3")
            for k in range(NCHUNK):
                s = scr[k % 2]
                nc.sync.dma_start(out=s[:], in_=xs[3][:, k * CW : (k + 1) * CW])
                scan_chunk(s[:], 3, k, pmn3, pmx3)

            W = {}
            for c in range(3):
                W[c] = dpool.tile([128, F_FULL], f32, tag="W", name=f"W{c}")
                pmn = pt.tile([128, NCHUNK], f32, tag="pmn", name=f"pmn{c}")
                pmx = pt.tile([128, NCHUNK], f32, tag="pmx", name=f"pmx{c}")
                for k in range(NCHUNK):
                    ck = W[c][:, k * CW : (k + 1) * CW]
                    nc.sync.dma_start(out=ck, in_=xs[c][:, k * CW : (k + 1) * CW])
                    scan_chunk(ck, c, k, pmn, pmx)
                combine(c, pmn, pmx)
            combine(3, pmn3, pmx3)

            # ---------- Phase 2+3 interleaved: chain (1 pair lookahead) + units
            # boot
            D0 = s4("D0")
            nc.vector.tensor_sub(D0[:], mx0t[:], mn0t[:])
            Dse0 = s4("Dse0")
            nc.vector.tensor_scalar_add(Dse0[:], D0[:], EPS)
            sp0 = s4("sp0")
            nc.vector.reciprocal(sp0[:], Dse0[:])
            A = s4("A")
            nc.vector.tensor_scalar(A[:], sp0[:], -EPS, 1.0, Alu.mult, Alu.add)
            rgamma = sp0
            delta = mn0t

            state = {"A": A, "rgamma": rgamma, "delta": delta, "gmid": None, "dmid": None}

            def chain_layer_stats(l):
                t1 = s4("t1")
                nc.vector.tensor_add(t1[:], state["A"][:], cv("d2", l))
                eA = s4("eA")
                nc.vector.tensor_mul(eA[:], t1[:], t1[:])
                i_ = s4("i_")
                nc.vector.scalar_tensor_tensor(
                    i_[:], t1[:], 0.0, cv("nf", l), Alu.is_gt, Alu.mult
                )
                j = s4("j")
                nc.vector.tensor_scalar(j[:], i_[:], -1.0, 1.0, Alu.mult, Alu.add)
                mne = s4("mne")
                nc.vector.tensor_tensor(mne[:], eA[:], cv("e0", l), Alu.min)
                mn = s4("mn")
                nc.vector.tensor_mul(mn[:], mne[:], j[:])
                mx = s4("mx")
                nc.vector.tensor_tensor(mx[:], eA[:], cv("e0", l), Alu.max)
                spr = s4("spr")
                nc.vector.tensor_sub(spr[:], mx[:], mn[:])
                tg = s4("tg")
                nc.vector.tensor_mul(tg[:], spr[:], cv("g", l))
                E = s4("E")
                nc.vector.tensor_sub(E[:], mx[:], tg[:])
                tD = s4("tD")
                nc.vector.tensor_mul(tD[:], spr[:], cv("absa2", l))
                Dse = s4("Dse")
                nc.vector.tensor_scalar_add(Dse[:], tD[:], EPS)
                sp = s4("sp")
                nc.vector.reciprocal(sp[:], Dse[:])
                Anew = s4("Anew")
                nc.vector.tensor_scalar(Anew[:], sp[:], -EPS, 1.0, Alu.mult, Alu.add)
                state["A"] = Anew
                return E, Dse, sp

            def chain_A_layer(lA, av, bv):
                """scaled layer: writes alpha/beta; returns (gmid, dmid)."""
                E, Dse, sp = chain_layer_stats(lA)
                aspa = s4("aspa")
                nc.vector.tensor_mul(aspa[:], cv("absa2", lA), sp[:])
                w = s4("w")
                nc.scalar.activation(w[:], aspa[:], Act.Sqrt)
                w2 = s4("w2")
                nc.vector.tensor_mul(w2[:], w[:], w[:])
                raspa = s4("raspa")
                nc.vector.tensor_mul(raspa[:], Dse[:], cv("r_absa2", lA))
                gmu = s4("gmu")
                nc.vector.tensor_mul(gmu[:], w2[:], raspa[:])
                gmid = s4("gmid")
                nc.vector.tensor_mul(gmid[:], gmu[:], cv("sgn", lA))
                nc.vector.tensor_mul(av, w[:], state["rgamma"][:])
                tad = s4("tad")
                nc.vector.tensor_mul(tad[:], av, state["delta"][:])
                twd = s4("twd")
                nc.vector.tensor_mul(twd[:], w[:], cv("d2", lA))
                nc.vector.tensor_sub(bv, twd[:], tad[:])
                dmid = s4("dmid")
                nc.vector.tensor_mul(dmid[:], w2[:], E[:])
                return gmid, dmid

            def chain_unit_layer(l, gam_in, del_in, bbv):
                """unit layer: Z' = (Z + bb)^2 given input affine (gam, del).
                Writes bb; returns (gam_out, del_out)."""
                E2, Dse2, _ = chain_layer_stats(l)
                tbd = s4("tbd")
                nc.vector.tensor_mul(tbd[:], gam_in[:], cv("d2", l))
                nc.vector.tensor_sub(bbv, tbd[:], del_in[:])
                gm2 = s4("gm2")
                nc.vector.tensor_mul(gm2[:], gam_in[:], gam_in[:])
                tg2 = s4("tg2")
                nc.vector.tensor_mul(tg2[:], gm2[:], Dse2[:])
                gam = s4("gam")
                nc.vector.tensor_mul(gam[:], tg2[:], cv("r_a2", l))
                dele = s4("dele")
                nc.vector.tensor_mul(dele[:], gm2[:], E2[:])
                return gam, dele, gm2

            def chain_group(g):
                av = alphaT[:, g * 4 : g * 4 + 4]
                bv = betaT[:, g * 4 : g * 4 + 4]
                if g < 6:
                    lA = 3 * g
                    gmid, dmid = chain_A_layer(lA, av, bv)
                    bbB = bb2T[:, g * 8 + 0 : g * 8 + 8 : 2]
                    bbC = bb2T[:, g * 8 + 1 : g * 8 + 8 : 2]
                    gamB, delB, _ = chain_unit_layer(lA + 1, gmid, dmid, bbB)
                    gamC, delC, _ = chain_unit_layer(lA + 2, gamB, delB, bbC)
                    rg = s4("rg")
                    nc.vector.reciprocal(rg[:], gamC[:])
                    state["rgamma"] = rg
                    state["delta"] = delC
                else:
                    lA = 18
                    gmid, dmid = chain_A_layer(lA, av, bv)
                    E2, Dse2, _ = chain_layer_stats(19)
                    tbd = s4("tbd")
                    nc.vector.tensor_mul(tbd[:], gmid[:], cv("d2", 19))
                    nc.vector.tensor_sub(betabT[:], tbd[:], dmid[:])
                    gm2 = s4("gm2")
                    nc.vector.tensor_mul(gm2[:], gmid[:], gmid[:])
                    rgm2 = s4("rgm2")
                    nc.vector.reciprocal(rgm2[:], gm2[:])
                    a2c = s4("a2c")
                    nc.vector.tensor_mul(a2c[:], cv("absa2", 19), cv("sgn", 19))
                    nc.vector.tensor_mul(cf1T[:], a2c[:], rgm2[:])

            def unit(c, k, g):
                ck = W[c][:, k * CW : (k + 1) * CW]
                a_ap = alphaT[:, g * 4 + c : g * 4 + c + 1]
                b_ap = betaT[:, g * 4 + c : g * 4 + c + 1]
                if g < 6:
                    bb_pair = bb2T[:, g * 8 + 2 * c : g * 8 + 2 * c + 2]
                    if unit_engine_is_act(c, k, g):
                        nc.scalar.activation(ck, ck, Act.Square, bias=b_ap, scale=a_ap)
                        nc.scalar.activation(
                            ck, ck, Act.Square,
                            bias=bb2T[:, g * 8 + 2 * c : g * 8 + 2 * c + 1], scale=1.0,
                        )
                        nc.scalar.activation(
                            ck, ck, Act.Square,
                            bias=bb2T[:, g * 8 + 2 * c + 1 : g * 8 + 2 * c + 2], scale=1.0,
                        )
                    else:
                        nc.vector._custom_dve(
                            triple_op, out=ck, in0=ck, in1=bb_pair, s0=a_ap, s1=b_ap
                        )
                else:
                    bb_ap = betabT[:, c : c + 1]
                    if unit_engine_is_act(c, k, g):
                        nc.scalar.activation(ck, ck, Act.Square, bias=b_ap, scale=a_ap)
                        nc.scalar.activation(ck, ck, Act.Square, bias=bb_ap, scale=1.0)
                    else:
                        nc.vector._custom_dve(
                            pair_op, out=ck, in0=ck, in1=bb_ap, s0=a_ap, s1=b_ap
                        )

            def finish_chunk(c, k):
                ck = W[c][:, k * CW : (k + 1) * CW]
                cf1_ap = cf1T[:, c : c + 1]
                cf0_ap = cf0v[:, c : c + 1]
                if affine_engine_is_act(c, k):
                    nc.scalar.activation(ck, ck, Act.Identity, bias=cf0_ap, scale=cf1_ap)
                else:
                    nc.vector.tensor_scalar(ck, ck, cf1_ap, cf0_ap, Alu.mult, Alu.add)
                nc.sync.dma_start(out=ys[c][:, k * CW : (k + 1) * CW], in_=ck)

            # full chain upfront (ACT is idle during the head anyway; having all
            # group constants ready removes ordering constraints on the units)
            for g in range(N_GROUPS):
                chain_group(g)

            # ch0 chunk-major first: each chunk finishes (and frees its buffer
            # region for ch3's DMA, subtile-tracked) as early as possible
            for k in range(NCHUNK):
                for g in range(N_GROUPS):
                    unit(0, k, g)
                finish_chunk(0, k)

            # ---------- ch3 load starts as soon as ch0 chunks drain ----------
            W[3] = dpool.tile([128, F_FULL], f32, tag="W", name="W3")
            for k in range(NCHUNK):
                nc.sync.dma_start(
                    out=W[3][:, k * CW : (k + 1) * CW],
                    in_=xs[3][:, k * CW : (k + 1) * CW],
                )

            for g in range(N_GROUPS):
                for c in (1, 2):
                    for k in range(NCHUNK):
                        unit(c, k, g)
            for c in (1, 2):
                for k in range(NCHUNK):
                    finish_chunk(c, k)

            for k in range(NCHUNK):
                for g in range(N_GROUPS):
                    unit(3, k, g)
                finish_chunk(3, k)

    nc.compile()
    return nc


_NC_CACHE = {}


def _get_nc():
    if "full" not in _NC_CACHE:
        _NC_CACHE["full"] = build_nc()
    return _NC_CACHE["full"]


def host_coefs(w0, w1, w2):
    """Per-core coef arrays [128, NCOEF] (f32, broadcast over partitions)."""
    f = np.float32
    a2 = np.asarray(w2, dtype=f)
    a1 = np.asarray(w1, dtype=f)
    a0 = np.asarray(w0, dtype=f)
    sgn = np.where(a2 >= 0, f(1), f(-1)).astype(f)
    a2cl = (sgn * np.maximum(np.abs(a2), f(CLAMP))).astype(f)
    d2 = (a1 / a2cl / 2).astype(f)
    e0 = (d2 * d2).astype(f)
    nf = (d2 < 0).astype(f)
    g = (a2cl >= 0).astype(f)
    absa2 = np.abs(a2cl).astype(f)
    r_a2 = (f(1) / a2cl).astype(f)
    r_absa2 = (f(1) / absa2).astype(f)
    arrays = {
        "d2": d2, "e0": e0, "nf": nf, "g": g,
        "absa2": absa2, "r_a2": r_a2, "sgn": sgn, "r_absa2": r_absa2,
    }
    cf0 = (a0[N_LAYERS - 1] - a2cl[N_LAYERS - 1] * e0[N_LAYERS - 1]).astype(f)

    out = []
    for core in range(N_CORES):
        cols = slice(CH_PER_CORE * core, CH_PER_CORE * (core + 1))
        row = np.empty(NCOEF, dtype=f)
        for idx, nm in enumerate(_COEF_NAMES):
            arr = arrays[nm][:, cols]  # [NL, 4]
            row[idx * _NL4 : (idx + 1) * _NL4] = arr.reshape(-1)  # l*4+c
        row[len(_COEF_NAMES) * _NL4 :] = cf0[cols]
        out.append(np.ascontiguousarray(np.broadcast_to(row[None, :], (128, NCOEF))))
    return out


def shard_inputs(x, w0, w1, w2):
    x = np.ascontiguousarray(x, dtype=np.float32)
    coefs = host_coefs(w0, w1, w2)
    in_maps = []
    for k in range(N_CORES):
        cols = slice(CH_PER_CORE * k, CH_PER_CORE * (k + 1))
        xk = np.ascontiguousarray(x[:, cols].transpose(1, 0, 2, 3)).reshape(
            CH_PER_CORE, 128, F_FULL
        )
        in_maps.append({"xs": xk, "coef": coefs[k]})
    return in_maps


def unshard_output(results):
    out = np.empty((B, C, H, Wd), dtype=np.float32)
    for k in range(N_CORES):
        ysk = np.asarray(results[k]["ys"], dtype=np.float32).reshape(
            CH_PER_CORE, B, H, Wd
        )
        out[:, CH_PER_CORE * k : CH_PER_CORE * (k + 1)] = ysk.transpose(1, 0, 2, 3)
    return out


def run_sharded(in_maps, trace=False, trace_kwargs=None):
    _import_concourse()
    from concourse.bass_utils import run_bass_kernel_spmd

    nc = _get_nc()
    return run_bass_kernel_spmd(
        nc,
        in_maps,
        core_ids=list(range(N_CORES)),
        trace=trace,
        **(trace_kwargs or {}),
    )


def kernel(x, w0, w1, w2):
    in_maps = shard_inputs(x, w0, w1, w2)
    res = run_sharded(in_maps)
    return unshard_output(res.results)
